# revision 1
# baseline (speedup 1.0000x reference)
"""MPNN (2x NNConv + BN + global mean pool + MLP) on 8 Trainium2 cores.

Strategy (node-sharded message passing):
  * Never materialize We=[E,in_c,out_c].  msg[e] = (z[e] (x) xs[e]) @ W2r
    where z=relu(ea@W1+b1), xs=x[src], W2r = reshape of W2.  Since the
    segment-sum over dst commutes with the (shared) @W2r, we scatter the
    per-edge outer products u[e]=(z (x) xs_scaled) into per-node U first,
    then do ONE matmul per node tile:  agg = U @ W2r  (3x fewer MACs).
  * Nodes are bin-packed into 128-node windows balanced by edge count;
    each core owns 10 windows.  Edges are grouped per window (padded to
    T tiles of 128).  Scatter = one-hot matmul on the PE accumulating
    U^T chunks directly in PSUM (no transposes needed anywhere in the
    scatter->agg path).  Matmul operands in bf16, PSUM accumulates f32.
  * BN1 is folded into the layer-2 gather: h1 is AllGathered pre-BN
    (with the BN stats riding along as 2 extra rows), and the per-
    feature affine is applied to the gathered xs.  BN2+pool commute
    (sum of 1/gc over a graph's nodes is 1), so pooling runs on the
    un-normalized relu output and BN2 is applied after the final
    AllReduce, which also carries the BN2 stats.  Two collectives total.
"""

import sys

import numpy as np

try:
    import concourse.bass as bass  # noqa
except Exception:  # pragma: no cover
    sys.path.insert(0, "/opt/trn_rl_repo")

import ml_dtypes
import concourse.bacc as bacc
import concourse.bass as bass
import concourse.mybir as mybir
import concourse.tile as tile
from concourse.bass import IndirectOffsetOnAxis
from concourse.bass_utils import run_bass_kernel_spmd
from concourse.masks import make_identity

P = 128
NCORES = 8
N = 10000
E = 30000
NG = 256
IN_C = 16
EDGE_C = 8
KH = 32  # edge-MLP hidden width
H1 = 64  # conv1 out channels
H2 = 128  # conv2 out channels
WPC = 10  # windows per core
NPADC = WPC * P  # padded nodes per core (1280)
NSTR = NPADC + 2  # h1 slice rows per core incl. 2 BN-stats rows
EPS = 1e-5
f32 = mybir.dt.float32
bf16 = mybir.dt.bfloat16
i32 = mybir.dt.int32

U1 = KH * IN_C  # 512
U1E = 640  # padded to 5 chunks of 128 (u | xs | zeros)
U2 = KH * H1  # 2048
U2E = 2176  # padded to 17 chunks of 128 (u | xs | zeros)


# --------------------------------------------------------------------------
# host-side preprocessing: index/layout work only
# --------------------------------------------------------------------------
def _preprocess(x, edge_index, edge_attr, batch):
    import heapq

    src = np.asarray(edge_index[0], dtype=np.int64)
    dst = np.asarray(edge_index[1], dtype=np.int64)
    deg = np.bincount(dst, minlength=N).astype(np.int64)

    # ---- bin-pack nodes into NCORES*WPC windows of exactly <=128 nodes,
    # balancing per-window edge counts (LPT greedy) ----
    NW = NCORES * WPC
    order = np.argsort(-deg, kind="stable")
    wsum = np.zeros(NW, dtype=np.int64)
    wcnt = np.zeros(NW, dtype=np.int64)
    win_of = np.empty(N, dtype=np.int64)
    slot_of = np.empty(N, dtype=np.int64)
    heap = [(0, w) for w in range(NW)]
    heapq.heapify(heap)
    for n in order:
        while True:
            _, w = heapq.heappop(heap)
            if wcnt[w] < P:
                break
        win_of[n] = w
        slot_of[n] = wcnt[w]
        wcnt[w] += 1
        wsum[w] += deg[n]
        if wcnt[w] < P:
            heapq.heappush(heap, (int(wsum[w]), w))

    T = max(1, int(-(-int(wsum.max()) // P)))  # tiles (of 128 edges) per window
    ES = WPC * T * P  # edge slots per core

    core_of = win_of // WPC
    lpos = (win_of % WPC) * P + slot_of

    # ---- per-edge placement ----
    ew = win_of[dst]
    eorder = np.argsort(ew, kind="stable")
    inv_cnt = 1.0 / np.maximum(deg, 1).astype(np.float32)

    ea_s = np.zeros((NCORES, ES, EDGE_C), dtype=np.float32)
    ones_s = np.zeros((NCORES, ES), dtype=np.float32)
    srcx_s = np.zeros((NCORES, ES), dtype=np.int32)
    srch_s = np.zeros((NCORES, ES), dtype=np.int32)
    dstrel_s = np.full((NCORES, ES), -1.0, dtype=np.float32)
    icnt_s = np.zeros((NCORES, ES), dtype=np.float32)

    ew_sorted = ew[eorder]
    starts = np.searchsorted(ew_sorted, np.arange(NW))
    ends = np.searchsorted(ew_sorted, np.arange(NW) + 1)
    ea_np = np.asarray(edge_attr, dtype=np.float32)
    for w in range(NW):
        es = eorder[starts[w] : ends[w]]
        c = w // WPC
        base = (w % WPC) * T * P
        k = len(es)
        assert k <= T * P
        sl = slice(base, base + k)
        ea_s[c, sl] = ea_np[es]
        ones_s[c, sl] = 1.0
        srcx_s[c, sl] = src[es]
        srch_s[c, sl] = core_of[src[es]] * NSTR + lpos[src[es]]
        dstrel_s[c, sl] = slot_of[dst[es]]
        icnt_s[c, sl] = inv_cnt[dst[es]]

    eaT = np.concatenate(
        [np.transpose(ea_s, (0, 2, 1)), ones_s[:, None, :]], axis=1
    ).astype(np.float32)

    # ---- per-node per-core tables ----
    batch = np.asarray(batch, dtype=np.int64)
    gcnt = np.bincount(batch, minlength=NG).astype(np.int64)
    igc_node = (1.0 / np.maximum(gcnt, 1).astype(np.float32))[batch]
    gmask = (gcnt > 0).astype(np.float32)

    xT_s = np.zeros((NCORES, IN_C, NPADC), dtype=np.float32)
    batch_s = np.full((NCORES, NPADC), -1.0, dtype=np.float32)
    igc_s = np.zeros((NCORES, NPADC), dtype=np.float32)
    vmask_s = np.zeros((NCORES, NPADC), dtype=np.float32)
    x_np = np.asarray(x, dtype=np.float32)
    for c in range(NCORES):
        m = core_of == c
        xT_s[c][:, lpos[m]] = x_np[m].T
        batch_s[c][lpos[m]] = batch[m].astype(np.float32)
        igc_s[c][lpos[m]] = igc_node[m]
        vmask_s[c][lpos[m]] = 1.0

    return dict(
        T=T, ES=ES, eaT=eaT, srcx=srcx_s, srch=srch_s, dstrel=dstrel_s,
        icnt=icnt_s, xT=xT_s, batchrel=batch_s, igc=igc_s, vmask=vmask_s,
        gmask=gmask,
    )


def _weights(p):
    w = {}
    w["W1a1"] = np.concatenate([p["nn1_W1"], p["nn1_b1"][None, :]], 0).astype(np.float32)
    w["W1a2"] = np.concatenate([p["nn2_W1"], p["nn2_b1"][None, :]], 0).astype(np.float32)
    wp1 = np.zeros((U1E, H1), dtype=np.float32)
    wp1[:U1] = p["nn1_W2"].reshape(KH, IN_C, H1).reshape(U1, H1)
    wp1[U1 : U1 + IN_C] = p["nn1_b2"].reshape(IN_C, H1)
    w["Wp1"] = wp1
    wp2 = np.zeros((U2E, H2), dtype=np.float32)
    wp2[:U2] = p["nn2_W2"].reshape(KH, H1, H2).reshape(U2, H2)
    wp2[U2 : U2 + H1] = p["nn2_b2"].reshape(H1, H2)
    w["Wp2"] = wp2
    w["root1"] = np.asarray(p["root1"], np.float32)
    w["root2"] = np.asarray(p["root2"], np.float32)
    w["bias1r"] = np.asarray(p["bias1"], np.float32)[None, :]
    w["bias2r"] = np.asarray(p["bias2"], np.float32)[None, :]
    w["bng1r"] = np.asarray(p["bn1_g"], np.float32)[None, :]
    w["bnb1r"] = np.asarray(p["bn1_b"], np.float32)[None, :]
    w["bng2"] = np.asarray(p["bn2_g"], np.float32)[:, None]
    w["bnb2"] = np.asarray(p["bn2_b"], np.float32)[:, None]
    w["l1W"] = np.asarray(p["lin1_W"], np.float32)
    w["l1b"] = np.asarray(p["lin1_b"], np.float32)[:, None]
    w["l2W"] = np.asarray(p["lin2_W"], np.float32)
    w["l2b"] = np.asarray(p["lin2_b"], np.float32)[None, :]
    w["iota128"] = np.broadcast_to(np.arange(P, dtype=np.float32), (P, P)).copy()
    w["iota256"] = np.broadcast_to(np.arange(NG, dtype=np.float32), (P, NG)).copy()
    w["onesP"] = np.ones((P, 1), dtype=np.float32)
    w["onesr"] = np.ones((1, P), dtype=np.float32)
    w["onesrb"] = np.ones((1, P), dtype=ml_dtypes.bfloat16)
    return w


# --------------------------------------------------------------------------
# device program (identical for all cores; per-core data comes via inputs)
# --------------------------------------------------------------------------
def build_program(T, ES):
    AL = mybir.AluOpType
    AF = mybir.ActivationFunctionType
    nc = bacc.Bacc("TRN2", target_bir_lowering=False, debug=False, num_devices=NCORES)

    def din(name, shape, dtype=f32):
        return nc.dram_tensor(name, shape, dtype, kind="ExternalInput").ap()

    eaT_d = din("eaT", [EDGE_C + 1, ES])
    srcx_d = din("srcx", [ES, 1], i32)
    srch_d = din("srch", [ES, 1], i32)
    dstrel_d = din("dstrel", [ES, 1])
    icnt_d = din("icnt", [ES, 1])
    x_d = din("x", [N, IN_C])
    xT_d = din("xT", [IN_C, NPADC], bf16)
    batch_d = din("batchrel", [NPADC, 1])
    igc_d = din("igc", [NPADC, 1])
    vmask_d = din("vmask", [NPADC, 1])
    W1a1_d = din("W1a1", [EDGE_C + 1, KH])
    W1a2_d = din("W1a2", [EDGE_C + 1, KH])
    Wp1_d = din("Wp1", [U1E, H1], bf16)
    Wp2_d = din("Wp2", [U2E, H2], bf16)
    root1_d = din("root1", [IN_C, H1], bf16)
    root2_d = din("root2", [H1, H2], bf16)
    bias1r_d = din("bias1r", [1, H1], bf16)
    bias2r_d = din("bias2r", [1, H2], bf16)
    bng1r_d = din("bng1r", [1, H1])
    bnb1r_d = din("bnb1r", [1, H1])
    bng2_d = din("bng2", [H2, 1])
    bnb2_d = din("bnb2", [H2, 1])
    l1W_d = din("l1W", [H2, H1])
    l1b_d = din("l1b", [H1, 1])
    l2W_d = din("l2W", [H1, 1])
    l2b_d = din("l2b", [1, 1])
    iota128_d = din("iota128", [P, P])
    iota256_d = din("iota256", [P, NG])
    onesP_d = din("onesP", [P, 1])
    onesr_d = din("onesr", [1, P])
    onesrb_d = din("onesrb", [1, P], bf16)
    gmaskb_d = din("gmaskb", [P, NG])
    out_d = nc.dram_tensor("out", [1, NG], f32, kind="ExternalOutput").ap()

    NC1 = U1E // P  # 5 chunks
    NC2 = U2E // P  # 17 chunks

    from contextlib import ExitStack

    with tile.TileContext(nc) as tc, ExitStack() as pools:
        cst = pools.enter_context(tc.tile_pool(name="cst", bufs=1))
        sb = pools.enter_context(tc.tile_pool(name="sb", bufs=3))
        stash = pools.enter_context(tc.tile_pool(name="stash", bufs=WPC))
        pp_u = pools.enter_context(tc.tile_pool(name="pp_u", bufs=1, space="PSUM"))
        pp_pre = pools.enter_context(tc.tile_pool(name="pp_pre", bufs=1, space="PSUM"))
        pp_z = pools.enter_context(tc.tile_pool(name="pp_z", bufs=1, space="PSUM"))
        pp_g = pools.enter_context(tc.tile_pool(name="pp_g", bufs=1, space="PSUM"))
        dram = pools.enter_context(tc.tile_pool(name="dram", bufs=1, space="DRAM"))

        # ---- resident constants ----
        def load(shape, ap, name, dt=f32):
            t = cst.tile(shape, dt, tag=name)
            nc.sync.dma_start(out=t[:], in_=ap)
            return t

        ident = cst.tile([P, P], f32, tag="ident")
        make_identity(nc, ident[:])
        W1a1 = load([EDGE_C + 1, KH], W1a1_d[:], "W1a1")
        W1a2 = load([EDGE_C + 1, KH], W1a2_d[:], "W1a2")
        Wp1 = cst.tile([P, NC1, H1], bf16, tag="Wp1")
        nc.sync.dma_start(out=Wp1[:], in_=Wp1_d.rearrange("(c p) o -> p c o", p=P))
        Wp2 = cst.tile([P, NC2, H2], bf16, tag="Wp2")
        nc.sync.dma_start(out=Wp2[:], in_=Wp2_d.rearrange("(c p) o -> p c o", p=P))
        xT = load([IN_C, NPADC], xT_d[:], "xT", bf16)
        root1 = load([IN_C, H1], root1_d[:], "root1", bf16)
        root2 = load([H1, H2], root2_d[:], "root2", bf16)
        bias1r = load([1, H1], bias1r_d[:], "bias1r", bf16)
        bias2r = load([1, H2], bias2r_d[:], "bias2r", bf16)
        bng1r = load([1, H1], bng1r_d[:], "bng1r")
        bnb1r = load([1, H1], bnb1r_d[:], "bnb1r")
        bng2 = load([H2, 1], bng2_d[:], "bng2")
        bnb2 = load([H2, 1], bnb2_d[:], "bnb2")
        l1W = load([H2, H1], l1W_d[:], "l1W")
        l1b = load([H1, 1], l1b_d[:], "l1b")
        l2W = load([H1, 1], l2W_d[:], "l2W")
        l2b = load([1, 1], l2b_d[:], "l2b")
        iota128 = load([P, P], iota128_d[:], "iota128")
        iota256 = load([P, NG], iota256_d[:], "iota256")
        onesP = load([P, 1], onesP_d[:], "onesP")
        onesr = load([1, P], onesr_d[:], "onesr")
        onesrb = load([1, P], onesrb_d[:], "onesrb", bf16)
        gmaskb = load([P, NG], gmaskb_d[:], "gmaskb")
        # per-node tables as [slot(partition), window(free)]
        vmask = load([P, WPC], vmask_d.rearrange("(w s) o -> s (w o)", s=P), "vmask")
        igc = load([P, WPC], igc_d.rearrange("(w s) o -> s (w o)", s=P), "igc")
        batchrel = load([P, WPC], batch_d.rearrange("(w s) o -> s (w o)", s=P), "batchrel")

        stats_sb1 = cst.tile([1, 2 * H1], f32, tag="st1")
        stats_sb2 = cst.tile([H2, 2], f32, tag="st2")
        nc.vector.memset(stats_sb1[:], 0.0)
        nc.vector.memset(stats_sb2[:], 0.0)

        h1_slice = dram.tile([NSTR, H1], f32, tag="h1s")
        h1_full = dram.tile([NCORES * NSTR, H1], f32, tag="h1f")
        fin_loc = dram.tile([P, NG + 2], f32, tag="finl")
        fin_g = dram.tile([P, NG + 2], f32, tag="fing")

        groups = [list(range(NCORES))]
        gTp = [None]

        # ================= generic conv layer =================
        def conv_layer(layer, scale_bc=None, shift_bc=None, h1T_list=None):
            if layer == 1:
                inc, outc, UE, NCH, W1a, Wp = IN_C, H1, U1E, NC1, W1a1, Wp1
                uw = U1
            else:
                inc, outc, UE, NCH, W1a, Wp = H1, H2, U2E, NC2, W1a2, Wp2
                uw = U2
            pre_list = []
            for w in range(WPC):
                UT = pp_u.tile([P, NCH, P], f32, tag="ut")
                u_tiles = []
                oh_tiles = []
                for t3 in range(T):
                    t = w * T + t3
                    s0 = t * P
                    ea_t = sb.tile([EDGE_C + 1, P], f32, tag="ea")
                    nc.sync.dma_start(out=ea_t[:], in_=eaT_d[:, s0 : s0 + P])
                    idx = sb.tile([P, 1], i32, tag="idx")
                    nc.sync.dma_start(
                        out=idx[:],
                        in_=(srcx_d if layer == 1 else srch_d)[s0 : s0 + P, :],
                    )
                    drel = sb.tile([P, 1], f32, tag="drel")
                    nc.sync.dma_start(out=drel[:], in_=dstrel_d[s0 : s0 + P, :])
                    ic_t = sb.tile([P, 1], f32, tag="ic")
                    nc.sync.dma_start(out=ic_t[:], in_=icnt_d[s0 : s0 + P, :])

                    # z = relu(ea @ W1 + b1)
                    zp = pp_z.tile([P, KH], f32, tag="z")
                    nc.tensor.matmul(out=zp[:], lhsT=ea_t[:], rhs=W1a[:], start=True, stop=True)
                    z = sb.tile([P, KH], f32, tag="z_sb")
                    nc.vector.tensor_scalar_max(out=z[:], in0=zp[:], scalar1=0.0)

                    # gather xs; scale by 1/cnt(dst); layer 2 also applies
                    # the BN1 affine folded onto the gathered features
                    xs = sb.tile([P, inc], f32, tag="xs")
                    nc.gpsimd.indirect_dma_start(
                        out=xs[:],
                        out_offset=None,
                        in_=(x_d if layer == 1 else h1_full[:]),
                        in_offset=IndirectOffsetOnAxis(ap=idx[:, :1], axis=0),
                    )
                    xss = sb.tile([P, inc], f32, tag="xss")
                    if layer == 1:
                        nc.vector.tensor_scalar_mul(out=xss[:], in0=xs[:], scalar1=ic_t[:, :1])
                    else:
                        xsi = sb.tile([P, inc], f32, tag="xsi")
                        nc.vector.scalar_tensor_tensor(
                            out=xsi[:], in0=xs[:], scalar=ic_t[:, :1],
                            in1=scale_bc[:], op0=AL.mult, op1=AL.mult,
                        )
                        nc.vector.scalar_tensor_tensor(
                            out=xss[:], in0=shift_bc[:], scalar=ic_t[:, :1],
                            in1=xsi[:], op0=AL.mult, op1=AL.add,
                        )

                    # u = z (x) xss  -> [P, uw], plus xss block, plus zero pad
                    u = sb.tile([P, UE], bf16, tag="u")
                    nc.vector.tensor_tensor(
                        out=u[:, :uw].rearrange("p (k i) -> p k i", k=KH),
                        in0=z[:].unsqueeze(2).to_broadcast([P, KH, inc]),
                        in1=xss[:].unsqueeze(1).to_broadcast([P, KH, inc]),
                        op=AL.mult,
                    )
                    nc.scalar.copy(out=u[:, uw : uw + inc], in_=xss[:])
                    nc.gpsimd.memset(u[:, uw + inc :], 0.0)

                    # one-hot over window slots
                    oh = sb.tile([P, P], bf16, tag="oh")
                    nc.vector.tensor_scalar(
                        out=oh[:], in0=iota128[:], scalar1=drel[:, :1],
                        scalar2=None, op0=AL.is_equal,
                    )
                    u_tiles.append(u)
                    oh_tiles.append(oh)
                # scatter: chunk-outer so each PSUM accumulation group is
                # open-close before the next one starts
                for c in range(NCH):
                    for t3 in range(T):
                        nc.tensor.matmul(
                            out=UT[:, c, :],
                            lhsT=u_tiles[t3][:, c * P : (c + 1) * P],
                            rhs=oh_tiles[t3][:],
                            start=(t3 == 0),
                            stop=(t3 == T - 1),
                        )

                # ---- node phase for window w ----
                UTs = sb.tile([P, NCH, P], bf16, tag="uts")
                nc.vector.tensor_copy(out=UTs[:], in_=UT[:])
                pre = pp_pre.tile([P, outc], f32, tag="pre")
                for c in range(NCH):
                    nc.tensor.matmul(
                        out=pre[:], lhsT=UTs[:, c, :], rhs=Wp[:, c, :],
                        start=(c == 0), stop=False,
                    )
                if layer == 1:
                    nc.tensor.matmul(
                        out=pre[:], lhsT=xT[:, w * P : (w + 1) * P], rhs=root1[:],
                        start=False, stop=False,
                    )
                else:
                    nc.tensor.matmul(
                        out=pre[:], lhsT=h1T_list[w][:], rhs=root2[:],
                        start=False, stop=False,
                    )
                nc.tensor.matmul(
                    out=pre[:], lhsT=onesrb[:], rhs=(bias1r if layer == 1 else bias2r)[:],
                    start=False, stop=True,
                )
                # relu then mask out pad nodes
                if layer == 1:
                    pre_sb = stash.tile([P, outc], f32, tag="pre1")
                else:
                    pre_sb = sb.tile([P, outc], f32, tag="pre2")
                nc.vector.tensor_scalar(
                    out=pre_sb[:], in0=pre[:], scalar1=0.0,
                    scalar2=vmask[:, w : w + 1], op0=AL.max, op1=AL.mult,
                )
                sq = sb.tile([P, outc], f32, tag="sq")
                nc.scalar.activation(out=sq[:], in_=pre_sb[:], func=AF.Square)
                if layer == 1:
                    pre_list.append(pre_sb)
                    # pre-BN h1 rows go straight out for the AllGather
                    nc.sync.dma_start(
                        out=h1_slice[w * P : (w + 1) * P, :], in_=pre_sb[:]
                    )
                    # bn stats as one row [1, 2*H1]: [sum | sumsq]
                    stp = pp_pre.tile([1, 2 * H1], f32, tag="pre")
                    nc.tensor.matmul(out=stp[0:1, :H1], lhsT=onesP[:], rhs=pre_sb[:], start=True, stop=True)
                    nc.tensor.matmul(out=stp[0:1, H1:], lhsT=onesP[:], rhs=sq[:], start=True, stop=True)
                    nc.vector.tensor_add(out=stats_sb1[:], in0=stats_sb1[:], in1=stp[:])
                else:
                    # bn stats as cols [H2, 2]
                    stp = pp_pre.tile([H2, 2], f32, tag="pre")
                    nc.tensor.matmul(out=stp[:, 0:1], lhsT=pre_sb[:], rhs=onesP[:], start=True, stop=True)
                    nc.tensor.matmul(out=stp[:, 1:2], lhsT=sq[:], rhs=onesP[:], start=True, stop=True)
                    nc.vector.tensor_add(out=stats_sb2[:], in0=stats_sb2[:], in1=stp[:])
                    # pool the un-normalized output (BN2 applied post-AR)
                    h2p = sb.tile([P, H2], bf16, tag="h2p")
                    nc.vector.tensor_scalar_mul(out=h2p[:], in0=pre_sb[:], scalar1=igc[:, w : w + 1])
                    ohg = sb.tile([P, NG], bf16, tag="ohg")
                    nc.vector.tensor_scalar(
                        out=ohg[:], in0=iota256[:], scalar1=batchrel[:, w : w + 1],
                        scalar2=None, op0=AL.is_equal,
                    )
                    nc.tensor.matmul(
                        out=gTp[0][:], lhsT=h2p[:], rhs=ohg[:],
                        start=(w == 0), stop=(w == WPC - 1),
                    )
            return pre_list

        # ======================= layer 1 =======================
        pre1 = conv_layer(1)
        # stats ride along with the h1 AllGather as 2 extra rows
        nc.sync.dma_start(out=h1_slice[NPADC : NPADC + 1, :], in_=stats_sb1[:, :H1])
        nc.sync.dma_start(out=h1_slice[NPADC + 1 : NSTR, :], in_=stats_sb1[:, H1:])
        nc.gpsimd.collective_compute(
            "AllGather", mybir.AluOpType.bypass, replica_groups=groups,
            ins=[h1_slice.opt()], outs=[h1_full.opt()],
        )
        # reduce the 8 cores' stats rows (all on partition 0)
        sgat = sb.tile([1, NCORES, 2 * H1], f32, tag="sgat")
        stat_rows = h1_full[:].rearrange("(c r) f -> c r f", r=NSTR)[
            :, NPADC : NPADC + 2, :
        ]
        nc.sync.dma_start(
            out=sgat[:], in_=stat_rows.rearrange("c r f -> c (r f)").unsqueeze(0)
        )
        acc = sb.tile([1, 2 * H1], f32, tag="sacc")
        nc.vector.tensor_copy(out=acc[:], in_=sgat[:, 0, :])
        for c in range(1, NCORES):
            nc.vector.tensor_add(out=acc[:], in0=acc[:], in1=sgat[:, c, :])
        # bn1 coeffs (row orientation [1, H1])
        mu = sb.tile([1, H1], f32, tag="mu")
        nc.vector.tensor_scalar_mul(out=mu[:], in0=acc[:, :H1], scalar1=1.0 / N)
        va = sb.tile([1, H1], f32, tag="va")
        nc.vector.tensor_scalar_mul(out=va[:], in0=acc[:, H1:], scalar1=1.0 / N)
        musq = sb.tile([1, H1], f32, tag="musq")
        nc.vector.tensor_mul(out=musq[:], in0=mu[:], in1=mu[:])
        nc.vector.tensor_sub(out=va[:], in0=va[:], in1=musq[:])
        nc.vector.tensor_scalar_add(out=va[:], in0=va[:], scalar1=EPS)
        sd = sb.tile([1, H1], f32, tag="sd")
        nc.scalar.activation(out=sd[:], in_=va[:], func=AF.Sqrt)
        rs = sb.tile([1, H1], f32, tag="rs")
        nc.vector.reciprocal(out=rs[:], in_=sd[:])
        sc_r = sb.tile([1, H1], f32, tag="sc_r")
        nc.vector.tensor_mul(out=sc_r[:], in0=rs[:], in1=bng1r[:])
        sh_r = sb.tile([1, H1], f32, tag="sh_r")
        nc.vector.tensor_mul(out=sh_r[:], in0=mu[:], in1=sc_r[:])
        nc.vector.tensor_sub(out=sh_r[:], in0=bnb1r[:], in1=sh_r[:])
        # broadcast [P, H1] tiles for the folded gather affine
        scp = pp_pre.tile([P, H1], f32, tag="pre")
        nc.tensor.matmul(out=scp[:], lhsT=onesr[:], rhs=sc_r[:], start=True, stop=True)
        scale_bc = cst.tile([P, H1], f32, tag="scale_bc")
        nc.vector.tensor_copy(out=scale_bc[:], in_=scp[:])
        shp = pp_pre.tile([P, H1], f32, tag="pre")
        nc.tensor.matmul(out=shp[:], lhsT=onesr[:], rhs=sh_r[:], start=True, stop=True)
        shift_bc = cst.tile([P, H1], f32, tag="shift_bc")
        nc.vector.tensor_copy(out=shift_bc[:], in_=shp[:])
        # column coeffs [H1, 1] for the transposed h1 (root2 operand)
        sccp = pp_pre.tile([H1, 1], f32, tag="pre")
        nc.tensor.transpose(out=sccp[:], in_=sc_r[:], identity=ident[:1, :1])
        sc_c = sb.tile([H1, 1], f32, tag="sc_c")
        nc.vector.tensor_copy(out=sc_c[:], in_=sccp[:])
        shcp = pp_pre.tile([H1, 1], f32, tag="pre")
        nc.tensor.transpose(out=shcp[:], in_=sh_r[:], identity=ident[:1, :1])
        sh_c = sb.tile([H1, 1], f32, tag="sh_c")
        nc.vector.tensor_copy(out=sh_c[:], in_=shcp[:])
        # normalized h1^T per window (only consumer: root2 matmul)
        h1T_list = []
        for w in range(WPC):
            tp = pp_u.tile([H1, P], f32, tag="ut")
            nc.tensor.transpose(out=tp[:], in_=pre1[w][:], identity=ident[:])
            h1T = stash.tile([H1, P], bf16, tag="h1T")
            nc.vector.tensor_scalar(
                out=h1T[:], in0=tp[:], scalar1=sc_c[:, :1], scalar2=sh_c[:, :1],
                op0=AL.mult, op1=AL.add,
            )
            h1T_list.append(h1T)

        # ======================= layer 2 =======================
        gTp[0] = pp_g.tile([P, NG], f32, tag="gtp", name="gtp")
        conv_layer(2, scale_bc, shift_bc, h1T_list)

        # one final AllReduce carries pooled graph features + BN2 stats
        fin_sb = sb.tile([P, NG + 2], f32, tag="fin")
        nc.vector.tensor_copy(out=fin_sb[:, :NG], in_=gTp[0][:])
        nc.vector.tensor_copy(out=fin_sb[:, NG : NG + 2], in_=stats_sb2[:])
        nc.sync.dma_start(out=fin_loc[:], in_=fin_sb[:])
        nc.gpsimd.collective_compute(
            "AllReduce", mybir.AluOpType.add, replica_groups=groups,
            ins=[fin_loc.opt()], outs=[fin_g.opt()],
        )
        fin = sb.tile([P, NG + 2], f32, tag="fin2")
        nc.sync.dma_start(out=fin[:], in_=fin_g[:])
        # bn2 coeffs (column orientation [H2, 1])
        mu2 = sb.tile([H2, 1], f32, tag="mu2")
        nc.vector.tensor_scalar_mul(out=mu2[:], in0=fin[:, NG : NG + 1], scalar1=1.0 / N)
        va2 = sb.tile([H2, 1], f32, tag="va2")
        nc.vector.tensor_scalar_mul(out=va2[:], in0=fin[:, NG + 1 : NG + 2], scalar1=1.0 / N)
        musq2 = sb.tile([H2, 1], f32, tag="musq2")
        nc.vector.tensor_mul(out=musq2[:], in0=mu2[:], in1=mu2[:])
        nc.vector.tensor_sub(out=va2[:], in0=va2[:], in1=musq2[:])
        nc.vector.tensor_scalar_add(out=va2[:], in0=va2[:], scalar1=EPS)
        sd2 = sb.tile([H2, 1], f32, tag="sd2")
        nc.scalar.activation(out=sd2[:], in_=va2[:], func=AF.Sqrt)
        rs2 = sb.tile([H2, 1], f32, tag="rs2")
        nc.vector.reciprocal(out=rs2[:], in_=sd2[:])
        sc2 = sb.tile([H2, 1], f32, tag="sc2")
        nc.vector.tensor_mul(out=sc2[:], in0=rs2[:], in1=bng2[:])
        sh2 = sb.tile([H2, 1], f32, tag="sh2")
        nc.vector.tensor_mul(out=sh2[:], in0=mu2[:], in1=sc2[:])
        nc.vector.tensor_sub(out=sh2[:], in0=bnb2[:], in1=sh2[:])
        # g = sc2 * g_raw + sh2 * gmask   (BN2 folded through the pool)
        gt = sb.tile([P, NG], f32, tag="gt")
        nc.vector.tensor_scalar_mul(out=gt[:], in0=fin[:, :NG], scalar1=sc2[:, :1])
        nc.vector.scalar_tensor_tensor(
            out=gt[:], in0=gmaskb[:], scalar=sh2[:, :1], in1=gt[:],
            op0=AL.mult, op1=AL.add,
        )

        # ======================= final MLP =======================
        l1p = pp_pre.tile([H1, NG], f32, tag="pre")
        nc.tensor.matmul(out=l1p[:], lhsT=l1W[:], rhs=gt[:], start=True, stop=True)
        hl = sb.tile([H1, NG], f32, tag="hl")
        nc.vector.tensor_scalar(
            out=hl[:], in0=l1p[:], scalar1=l1b[:, :1], scalar2=0.0,
            op0=AL.add, op1=AL.max,
        )
        l2p = pp_z.tile([1, NG], f32, tag="z")
        nc.tensor.matmul(out=l2p[:], lhsT=l2W[:], rhs=hl[:], start=True, stop=True)
        osb = sb.tile([1, NG], f32, tag="osb")
        nc.vector.tensor_scalar_add(out=osb[:], in0=l2p[:], scalar1=l2b[:, :1])
        nc.sync.dma_start(out=out_d[:], in_=osb[:])

    nc.compile()
    return nc


_CACHE = {}


def _get_program(T, ES):
    key = (T, ES)
    if key not in _CACHE:
        _CACHE[key] = build_program(T, ES)
    return _CACHE[key]


def make_in_maps(inputs):
    pp = _preprocess(
        inputs["x"], inputs["edge_index"], inputs["edge_attr"], inputs["batch"]
    )
    w = _weights(inputs)
    bf = ml_dtypes.bfloat16
    shared = dict(
        x=np.ascontiguousarray(np.asarray(inputs["x"], np.float32)),
        W1a1=w["W1a1"], W1a2=w["W1a2"],
        Wp1=w["Wp1"].astype(bf), Wp2=w["Wp2"].astype(bf),
        root1=w["root1"].astype(bf), root2=w["root2"].astype(bf),
        bias1r=w["bias1r"].astype(bf), bias2r=w["bias2r"].astype(bf),
        bng1r=w["bng1r"], bnb1r=w["bnb1r"], bng2=w["bng2"], bnb2=w["bnb2"],
        l1W=w["l1W"], l1b=w["l1b"], l2W=w["l2W"], l2b=w["l2b"],
        iota128=w["iota128"], iota256=w["iota256"], onesP=w["onesP"],
        onesr=w["onesr"], onesrb=w["onesrb"],
        gmaskb=np.ascontiguousarray(
            np.broadcast_to(pp["gmask"], (P, NG)).astype(np.float32)
        ),
    )
    in_maps = []
    for c in range(NCORES):
        m = dict(shared)
        m["eaT"] = np.ascontiguousarray(pp["eaT"][c])
        m["srcx"] = np.ascontiguousarray(pp["srcx"][c][:, None])
        m["srch"] = np.ascontiguousarray(pp["srch"][c][:, None])
        m["dstrel"] = np.ascontiguousarray(pp["dstrel"][c][:, None])
        m["icnt"] = np.ascontiguousarray(pp["icnt"][c][:, None])
        m["xT"] = np.ascontiguousarray(pp["xT"][c].astype(bf))
        m["batchrel"] = np.ascontiguousarray(pp["batchrel"][c][:, None])
        m["igc"] = np.ascontiguousarray(pp["igc"][c][:, None])
        m["vmask"] = np.ascontiguousarray(pp["vmask"][c][:, None])
        in_maps.append(m)
    return in_maps, pp["T"], pp["ES"]


def _run(inputs, trace=False):
    in_maps, T, ES = make_in_maps(inputs)
    nc = _get_program(T, ES)
    res = run_bass_kernel_spmd(
        nc, in_maps, core_ids=list(range(NCORES)), trace=trace
    )
    out = np.asarray(res.results[0]["out"][0], dtype=np.float32)
    return out, res


def kernel(**inputs):
    return _run(inputs)[0]



# revision 8
# speedup vs baseline: 1.2649x; 1.2649x over previous
"""MPNN (2x NNConv + BN + global mean pool + MLP) on 8 Trainium2 cores.

Strategy (node-sharded message passing), v2:
  * Never materialize We=[E,in_c,out_c].  msg[e] = (z[e] (x) xs[e]) @ W2r
    where z=relu(ea@W1+b1), xs=x[src], W2r = reshape of W2.  Since the
    segment-sum over dst commutes with the (shared) @W2r, we scatter the
    per-edge outer products u[e]=(z (x) xs_scaled) into per-node U first,
    then do ONE matmul per node tile:  agg = U @ W2r.
  * Nodes are bin-packed into 128-node windows balanced by edge count;
    each core owns 10 windows.  Edges are grouped per window (padded to
    T tiles of 128).  Scatter = one-hot matmul on the PE accumulating
    U^T chunks in PSUM, pipelined per 128-wide chunk with the
    PSUM->SBUF cast (scalar engine) and the node matmul.
  * v2 layout changes vs v1: all per-edge tables are preloaded once as
    resident SBUF tiles (no per-tile DMAs); x[src] for layer 1 is
    gathered host-side (pure indexing) so layer 1 has no indirect DMAs;
    bias1/bias2 ride as an extra ones-row on the root operand; BN1 is
    folded into the layer-2 gather; BN2+pool commute so BN2 is applied
    after the final AllReduce.  Collectives use Shared outputs; the
    layer-2 edge-MLP z tiles / one-hots / h1^T transposes are computed
    during the h1 AllGather.
"""

import sys

import numpy as np

try:
    import concourse.bass as bass  # noqa
except Exception:  # pragma: no cover
    sys.path.insert(0, "/opt/trn_rl_repo")

import ml_dtypes
import concourse.bacc as bacc
import concourse.bass as bass
import concourse.mybir as mybir
import concourse.tile as tile
from concourse.bass import IndirectOffsetOnAxis
from concourse.bass_utils import run_bass_kernel_spmd
from concourse.masks import make_identity

P = 128
NCORES = 8
N = 10000
E = 30000
NG = 256
IN_C = 16
EDGE_C = 8
KH = 32  # edge-MLP hidden width
H1 = 64  # conv1 out channels
H2 = 128  # conv2 out channels
WPC = 10  # windows per core
NPADC = WPC * P  # padded nodes per core (1280)
NSTR = NPADC + 2  # h1 slice rows per core incl. 2 BN-stats rows
EPS = 1e-5
f32 = mybir.dt.float32
bf16 = mybir.dt.bfloat16
i32 = mybir.dt.int32

U1 = KH * IN_C  # 512
U1E = 640  # padded to 5 chunks of 128 (u | xs | zeros)
U2 = KH * H1  # 2048
U2E = 2176  # padded to 17 chunks of 128 (u | xs | zeros)


# --------------------------------------------------------------------------
# host-side preprocessing: index/layout work only
# --------------------------------------------------------------------------
def _preprocess(x, edge_index, edge_attr, batch):
    import heapq

    src = np.asarray(edge_index[0], dtype=np.int64)
    dst = np.asarray(edge_index[1], dtype=np.int64)
    deg = np.bincount(dst, minlength=N).astype(np.int64)

    # ---- bin-pack nodes into NCORES*WPC windows of exactly <=128 nodes,
    # balancing per-window edge counts (LPT greedy) ----
    NW = NCORES * WPC
    order = np.argsort(-deg, kind="stable")
    wsum = np.zeros(NW, dtype=np.int64)
    wcnt = np.zeros(NW, dtype=np.int64)
    win_of = np.empty(N, dtype=np.int64)
    slot_of = np.empty(N, dtype=np.int64)
    heap = [(0, w) for w in range(NW)]
    heapq.heapify(heap)
    for n in order:
        while True:
            _, w = heapq.heappop(heap)
            if wcnt[w] < P:
                break
        win_of[n] = w
        slot_of[n] = wcnt[w]
        wcnt[w] += 1
        wsum[w] += deg[n]
        if wcnt[w] < P:
            heapq.heappush(heap, (int(wsum[w]), w))

    T = max(1, int(-(-int(wsum.max()) // P)))  # tiles (of 128 edges) per window
    NT = WPC * T  # edge tiles per core
    ES = NT * P  # edge slots per core

    core_of = win_of // WPC
    lpos = (win_of % WPC) * P + slot_of

    # ---- per-edge placement ----
    ew = win_of[dst]
    eorder = np.argsort(ew, kind="stable")
    inv_cnt = 1.0 / np.maximum(deg, 1).astype(np.float32)

    ea_s = np.zeros((NCORES, ES, EDGE_C), dtype=np.float32)
    ones_s = np.zeros((NCORES, ES), dtype=np.float32)
    srcx_s = np.zeros((NCORES, ES), dtype=np.int64)
    srch_s = np.zeros((NCORES, ES), dtype=np.int32)
    dstrel_s = np.full((NCORES, ES), -1.0, dtype=np.float32)
    icnt_s = np.zeros((NCORES, ES), dtype=np.float32)

    ew_sorted = ew[eorder]
    starts = np.searchsorted(ew_sorted, np.arange(NW))
    ends = np.searchsorted(ew_sorted, np.arange(NW) + 1)
    ea_np = np.asarray(edge_attr, dtype=np.float32)
    for w in range(NW):
        es = eorder[starts[w] : ends[w]]
        c = w // WPC
        base = (w % WPC) * T * P
        k = len(es)
        assert k <= T * P
        sl = slice(base, base + k)
        ea_s[c, sl] = ea_np[es]
        ones_s[c, sl] = 1.0
        srcx_s[c, sl] = src[es]
        srch_s[c, sl] = (core_of[src[es]] * NSTR + lpos[src[es]]).astype(np.int32)
        dstrel_s[c, sl] = slot_of[dst[es]]
        icnt_s[c, sl] = inv_cnt[dst[es]]

    eaT = np.concatenate(
        [np.transpose(ea_s, (0, 2, 1)), ones_s[:, None, :]], axis=1
    ).astype(np.float32)

    # host gather of x[src] (pure indexing), tile-major [P, NT*IN_C]
    x_np = np.asarray(x, dtype=np.float32)
    xsrc = x_np[srcx_s.reshape(NCORES, NT, P)]  # [C, NT, P, IN_C]
    xsrc_s = np.ascontiguousarray(
        xsrc.transpose(0, 2, 1, 3).reshape(NCORES, P, NT * IN_C)
    )

    # per-edge tables in [P(slot-in-tile), NT] layout
    def t_major(a):
        return np.ascontiguousarray(
            a.reshape(NCORES, NT, P).transpose(0, 2, 1)
        )

    srch_t = t_major(srch_s)
    drel_t = t_major(dstrel_s)
    icnt_t = t_major(icnt_s)

    # ---- per-node per-core tables ----
    batch = np.asarray(batch, dtype=np.int64)
    gcnt = np.bincount(batch, minlength=NG).astype(np.int64)
    igc_node = (1.0 / np.maximum(gcnt, 1).astype(np.float32))[batch]
    gmask = (gcnt > 0).astype(np.float32)

    xTa_s = np.zeros((NCORES, IN_C + 1, NPADC), dtype=np.float32)
    xTa_s[:, IN_C, :] = 1.0  # ones row for fused bias
    batch_s = np.full((NCORES, P, WPC), -1.0, dtype=np.float32)
    igc_s = np.zeros((NCORES, P, WPC), dtype=np.float32)
    vmask_s = np.zeros((NCORES, P, WPC), dtype=np.float32)
    for c in range(NCORES):
        m = core_of == c
        lp = lpos[m]
        xTa_s[c][:IN_C, lp] = x_np[m].T
        batch_s[c][lp % P, lp // P] = batch[m].astype(np.float32)
        igc_s[c][lp % P, lp // P] = igc_node[m]
        vmask_s[c][lp % P, lp // P] = 1.0

    return dict(
        T=T, ES=ES, NT=NT, eaT=eaT, srch=srch_t, drel=drel_t, icnt=icnt_t,
        xsrc=xsrc_s, xTa=xTa_s, batchrel=batch_s, igc=igc_s, vmask=vmask_s,
        gmask=gmask,
    )


def _weights(p):
    w = {}
    w["W1a1"] = np.concatenate([p["nn1_W1"], p["nn1_b1"][None, :]], 0).astype(np.float32)
    w["W1a2"] = np.concatenate([p["nn2_W1"], p["nn2_b1"][None, :]], 0).astype(np.float32)
    wp1 = np.zeros((U1E, H1), dtype=np.float32)
    wp1[:U1] = p["nn1_W2"].reshape(KH, IN_C, H1).reshape(U1, H1)
    wp1[U1 : U1 + IN_C] = p["nn1_b2"].reshape(IN_C, H1)
    w["Wp1"] = wp1
    wp2 = np.zeros((U2E, H2), dtype=np.float32)
    wp2[:U2] = p["nn2_W2"].reshape(KH, H1, H2).reshape(U2, H2)
    wp2[U2 : U2 + H1] = p["nn2_b2"].reshape(H1, H2)
    w["Wp2"] = wp2
    w["root1a"] = np.concatenate(
        [np.asarray(p["root1"], np.float32), np.asarray(p["bias1"], np.float32)[None, :]], 0
    )
    w["root2a"] = np.concatenate(
        [np.asarray(p["root2"], np.float32), np.asarray(p["bias2"], np.float32)[None, :]], 0
    )
    w["bng1r"] = np.asarray(p["bn1_g"], np.float32)[None, :]
    w["bnb1r"] = np.asarray(p["bn1_b"], np.float32)[None, :]
    w["bng2"] = np.asarray(p["bn2_g"], np.float32)[:, None]
    w["bnb2"] = np.asarray(p["bn2_b"], np.float32)[:, None]
    w["l1W"] = np.asarray(p["lin1_W"], np.float32)
    w["l1b"] = np.asarray(p["lin1_b"], np.float32)[:, None]
    w["l2W"] = np.asarray(p["lin2_W"], np.float32)
    w["l2b"] = np.asarray(p["lin2_b"], np.float32)[None, :]
    w["iota128"] = np.broadcast_to(np.arange(P, dtype=np.float32), (P, P)).copy()
    w["iota256"] = np.broadcast_to(np.arange(NG, dtype=np.float32), (P, NG)).copy()
    w["onesP"] = np.ones((P, 1), dtype=np.float32)
    w["onesr"] = np.ones((1, P), dtype=np.float32)
    return w


# --------------------------------------------------------------------------
# device program (identical for all cores; per-core data comes via inputs)
# --------------------------------------------------------------------------
def build_program(T, ES):
    NT = WPC * T
    AL = mybir.AluOpType
    AF = mybir.ActivationFunctionType
    nc = bacc.Bacc("TRN2", target_bir_lowering=False, debug=False, num_devices=NCORES)

    def din(name, shape, dtype=f32):
        return nc.dram_tensor(name, shape, dtype, kind="ExternalInput").ap()

    eaT_d = din("eaT", [EDGE_C + 1, ES])
    srch_d = din("srch", [P, NT], i32)
    drel_d = din("drel", [P, NT])
    icnt_d = din("icnt", [P, NT])
    xsrc_d = din("xsrc", [P, NT * IN_C])
    xTa_d = din("xTa", [IN_C + 1, NPADC], bf16)
    batch_d = din("batchrel", [P, WPC])
    igc_d = din("igc", [P, WPC])
    vmask_d = din("vmask", [P, WPC])
    W1a1_d = din("W1a1", [EDGE_C + 1, KH])
    W1a2_d = din("W1a2", [EDGE_C + 1, KH])
    Wp1_d = din("Wp1", [U1E, H1], bf16)
    Wp2_d = din("Wp2", [U2E, H2], bf16)
    root1a_d = din("root1a", [IN_C + 1, H1], bf16)
    root2a_d = din("root2a", [H1 + 1, H2], bf16)
    bng1r_d = din("bng1r", [1, H1])
    bnb1r_d = din("bnb1r", [1, H1])
    bng2_d = din("bng2", [H2, 1])
    bnb2_d = din("bnb2", [H2, 1])
    l1W_d = din("l1W", [H2, H1])
    l1b_d = din("l1b", [H1, 1])
    l2W_d = din("l2W", [H1, 1])
    l2b_d = din("l2b", [1, 1])
    iota128_d = din("iota128", [P, P], bf16)
    iota256_d = din("iota256", [P, NG], bf16)
    onesP_d = din("onesP", [P, 1])
    onesr_d = din("onesr", [1, P])
    gmaskb_d = din("gmaskb", [P, NG])
    out_d = nc.dram_tensor("out", [1, NG], f32, kind="ExternalOutput").ap()

    NC1 = U1E // P  # 5 chunks
    NC2 = U2E // P  # 17 chunks

    from contextlib import ExitStack

    with tile.TileContext(nc) as tc, ExitStack() as pools:
        cst = pools.enter_context(tc.tile_pool(name="cst", bufs=1))
        sb = pools.enter_context(tc.tile_pool(name="sb", bufs=3))
        stash = pools.enter_context(tc.tile_pool(name="stash", bufs=WPC))
        pp_u = pools.enter_context(tc.tile_pool(name="pp_u", bufs=1, space="PSUM"))
        pp_pre = pools.enter_context(tc.tile_pool(name="pp_pre", bufs=1, space="PSUM"))
        pp_z = pools.enter_context(tc.tile_pool(name="pp_z", bufs=1, space="PSUM"))
        pp_m = pools.enter_context(tc.tile_pool(name="pp_m", bufs=2, space="PSUM"))
        pp_g = pools.enter_context(tc.tile_pool(name="pp_g", bufs=1, space="PSUM"))
        dram = pools.enter_context(tc.tile_pool(name="dram", bufs=1, space="DRAM"))

        # ---- resident constants (spread initial DMAs across engine queues) ----
        _eng = [nc.sync, nc.scalar, nc.gpsimd]
        _ei = [0]

        def load(shape, ap, name, dt=f32):
            t = cst.tile(shape, dt, tag=name, name=name)
            _eng[_ei[0] % len(_eng)].dma_start(out=t[:], in_=ap)
            _ei[0] += 1
            return t

        ident = cst.tile([P, P], f32, tag="ident")
        make_identity(nc, ident[:])
        W1a1 = load([EDGE_C + 1, KH], W1a1_d[:], "W1a1")
        W1a2 = load([EDGE_C + 1, KH], W1a2_d[:], "W1a2")
        Wp1 = cst.tile([P, NC1, H1], bf16, tag="Wp1")
        nc.sync.dma_start(out=Wp1[:], in_=Wp1_d.rearrange("(c p) o -> p c o", p=P))
        Wp2 = cst.tile([P, NC2, H2], bf16, tag="Wp2")
        nc.scalar.dma_start(out=Wp2[:], in_=Wp2_d.rearrange("(c p) o -> p c o", p=P))
        eaT = load([EDGE_C + 1, ES], eaT_d[:], "eaT")
        srch = load([P, NT], srch_d[:], "srch", i32)
        drel = load([P, NT], drel_d[:], "drel")
        icnt = load([P, NT], icnt_d[:], "icnt")
        xsrc = load([P, NT * IN_C], xsrc_d[:], "xsrc")
        xTa = load([IN_C + 1, NPADC], xTa_d[:], "xTa", bf16)
        root1a = load([IN_C + 1, H1], root1a_d[:], "root1a", bf16)
        root2a = load([H1 + 1, H2], root2a_d[:], "root2a", bf16)
        bng1r = load([1, H1], bng1r_d[:], "bng1r")
        bnb1r = load([1, H1], bnb1r_d[:], "bnb1r")
        bng2 = load([H2, 1], bng2_d[:], "bng2")
        bnb2 = load([H2, 1], bnb2_d[:], "bnb2")
        l1W = load([H2, H1], l1W_d[:], "l1W")
        l1b = load([H1, 1], l1b_d[:], "l1b")
        l2W = load([H1, 1], l2W_d[:], "l2W")
        l2b = load([1, 1], l2b_d[:], "l2b")
        iota128 = load([P, P], iota128_d[:], "iota128", bf16)
        iota256 = load([P, NG], iota256_d[:], "iota256", bf16)
        onesP = load([P, 1], onesP_d[:], "onesP")
        onesr = load([1, P], onesr_d[:], "onesr")
        gmaskb = load([P, NG], gmaskb_d[:], "gmaskb")
        vmask = load([P, WPC], vmask_d[:], "vmask")
        igc = load([P, WPC], igc_d[:], "igc")
        batchrel = load([P, WPC], batch_d[:], "batchrel")

        # stats accumulators (SBUF, f32)
        acc1 = cst.tile([P, H1], f32, tag="acc1")
        acc1q = cst.tile([P, H1], f32, tag="acc1q")
        acc2 = cst.tile([P, H2], f32, tag="acc2")
        acc2q = cst.tile([P, H2], f32, tag="acc2q")
        for a in (acc1, acc1q, acc2, acc2q):
            nc.vector.memset(a[:], 0.0)

        h1_slice = dram.tile([NSTR, H1], f32, tag="h1s")
        h1_full = dram.tile([NCORES * NSTR, H1], f32, tag="h1f", addr_space="Shared")
        fin_loc = dram.tile([P, NG + 2], f32, tag="finl")
        fin_g = dram.tile([P, NG + 2], f32, tag="fing", addr_space="Shared")

        groups = [list(range(NCORES))]
        gTp = [None]

        # stash for L2 precomputed per-tile z (bf16) and one-hots (bf16)
        z2_all = cst.tile([P, NT, KH], bf16, tag="z2all")
        oh2_all = cst.tile([P, NT, P], bf16, tag="oh2all")

        # ================= generic conv layer =================
        def conv_layer(layer, scale_bc=None, shift_bc=None, h1T_list=None):
            if layer == 1:
                inc, outc, UE, NCH, W1a, Wp = IN_C, H1, U1E, NC1, W1a1, Wp1
                uw, acc, accq = U1, acc1, acc1q
            else:
                inc, outc, UE, NCH, W1a, Wp = H1, H2, U2E, NC2, W1a2, Wp2
                uw, acc, accq = U2, acc2, acc2q
            pre_list = []
            # slot-rotated PSUM tiles (PSUM slots are bank-granular)
            UT = pp_u.tile([P, 4, P], f32, tag="ut")
            PRE = pp_pre.tile([P, 4, P], f32, tag="pre")
            ZP = pp_z.tile([P, 4, KH], f32, tag="z")
            for w in range(WPC):
                u_tiles = []
                oh_tiles = []
                for t3 in range(T):
                    t = w * T + t3
                    s0 = t * P
                    if layer == 1:
                        # z = relu(ea @ W1 + b1)  (bf16 out)
                        zp = ZP[:, t % 4, :]
                        nc.tensor.matmul(
                            out=zp[:], lhsT=eaT[:, s0 : s0 + P], rhs=W1a[:],
                            start=True, stop=True,
                        )
                        z = sb.tile([P, KH], bf16, tag="z_sb", bufs=4)
                        nc.vector.tensor_scalar_max(out=z[:], in0=zp[:], scalar1=0.0)
                        oh = sb.tile([P, P], bf16, tag="oh", bufs=4)
                        nc.vector.tensor_scalar(
                            out=oh[:], in0=iota128[:], scalar1=drel[:, t : t + 1],
                            scalar2=None, op0=AL.is_equal,
                        )
                        xss = sb.tile([P, inc], bf16, tag="xss1", bufs=4)
                        nc.vector.tensor_scalar_mul(
                            out=xss[:], in0=xsrc[:, t * inc : (t + 1) * inc],
                            scalar1=icnt[:, t : t + 1],
                        )
                    else:
                        z = z2_all[:, t, :]
                        oh = oh2_all[:, t, :]
                        # gather xs rows from the AllGathered h1
                        xs = sb.tile([P, inc], f32, tag="xs", bufs=4)
                        nc.gpsimd.indirect_dma_start(
                            out=xs[:],
                            out_offset=None,
                            in_=h1_full[:],
                            in_offset=IndirectOffsetOnAxis(ap=srch[:, t : t + 1], axis=0),
                        )
                        # xss = (xs*icnt)*bn_scale + bn_shift*icnt  (bf16 out)
                        xsi = sb.tile([P, inc], f32, tag="xsi", bufs=3)
                        nc.vector.scalar_tensor_tensor(
                            out=xsi[:], in0=xs[:], scalar=icnt[:, t : t + 1],
                            in1=scale_bc[:], op0=AL.mult, op1=AL.mult,
                        )
                        xss = sb.tile([P, inc], bf16, tag="xss2", bufs=4)
                        nc.vector.scalar_tensor_tensor(
                            out=xss[:], in0=shift_bc[:], scalar=icnt[:, t : t + 1],
                            in1=xsi[:], op0=AL.mult, op1=AL.add,
                        )

                    # u = [z (x) xss | xss | junk-zeroed]
                    u = sb.tile([P, UE], bf16, tag=f"u{layer}", bufs=4)
                    nc.vector.tensor_tensor(
                        out=u[:, :uw].rearrange("p (k i) -> p k i", k=KH),
                        in0=z.unsqueeze(2).to_broadcast([P, KH, inc]),
                        in1=xss.unsqueeze(1).to_broadcast([P, KH, inc]),
                        op=AL.mult,
                    )
                    nc.scalar.copy(out=u[:, uw : uw + inc], in_=xss[:])
                    nc.gpsimd.memset(u[:, uw + inc :], 0.0)
                    u_tiles.append(u)
                    oh_tiles.append(oh)

                # scatter / cast / node matmul pipelined per chunk
                pre = PRE[:, w % 4, :outc]
                for c in range(NCH):
                    UTc = UT[:, (w * NCH + c) % 4, :]
                    for t3 in range(T):
                        nc.tensor.matmul(
                            out=UTc[:],
                            lhsT=u_tiles[t3][:, c * P : (c + 1) * P],
                            rhs=oh_tiles[t3][:],
                            start=(t3 == 0),
                            stop=(t3 == T - 1),
                        )
                    UTs = sb.tile([P, P], bf16, tag="uts", bufs=3)
                    nc.scalar.copy(out=UTs[:], in_=UTc[:])
                    nc.tensor.matmul(
                        out=pre[:], lhsT=UTs[:], rhs=Wp[:, c, :outc],
                        start=(c == 0), stop=False,
                    )
                # root + bias (ones row) closes the accumulation
                if layer == 1:
                    nc.tensor.matmul(
                        out=pre[:], lhsT=xTa[:, w * P : (w + 1) * P], rhs=root1a[:],
                        start=False, stop=True,
                    )
                else:
                    nc.tensor.matmul(
                        out=pre[:], lhsT=h1T_list[w][:], rhs=root2a[:],
                        start=False, stop=True,
                    )
                # relu then mask out pad nodes
                if layer == 1:
                    pre_sb = stash.tile([P, outc], f32, tag="pre1")
                else:
                    pre_sb = sb.tile([P, outc], f32, tag="pre2")
                nc.vector.tensor_scalar(
                    out=pre_sb[:], in0=pre[:], scalar1=0.0,
                    scalar2=vmask[:, w : w + 1], op0=AL.max, op1=AL.mult,
                )
                sq = sb.tile([P, outc], f32, tag="sq")
                nc.scalar.activation(out=sq[:], in_=pre_sb[:], func=AF.Square)
                nc.vector.tensor_add(out=acc[:, :outc], in0=acc[:, :outc], in1=pre_sb[:])
                nc.vector.tensor_add(out=accq[:, :outc], in0=accq[:, :outc], in1=sq[:])
                if layer == 1:
                    pre_list.append(pre_sb)
                    # pre-BN h1 rows go straight out for the AllGather
                    nc.sync.dma_start(
                        out=h1_slice[w * P : (w + 1) * P, :], in_=pre_sb[:]
                    )
                else:
                    # pool the un-normalized output (BN2 applied post-AR)
                    h2p = sb.tile([P, H2], bf16, tag="h2p")
                    nc.vector.tensor_scalar_mul(out=h2p[:], in0=pre_sb[:], scalar1=igc[:, w : w + 1])
                    ohg = sb.tile([P, NG], bf16, tag="ohg")
                    nc.vector.tensor_scalar(
                        out=ohg[:], in0=iota256[:], scalar1=batchrel[:, w : w + 1],
                        scalar2=None, op0=AL.is_equal,
                    )
                    nc.tensor.matmul(
                        out=gTp[0][:], lhsT=h2p[:], rhs=ohg[:],
                        start=(w == 0), stop=(w == WPC - 1),
                    )
            return pre_list

        # ======================= layer 1 =======================
        pre1 = conv_layer(1)
        # bn1 stats: [1, 2*H1] = [sum | sumsq] reduced over partitions
        stp1 = pp_m.tile([1, 2 * H1], f32, tag="m")
        nc.tensor.matmul(out=stp1[0:1, :H1], lhsT=onesP[:], rhs=acc1[:], start=True, stop=True)
        nc.tensor.matmul(out=stp1[0:1, H1:], lhsT=onesP[:], rhs=acc1q[:], start=True, stop=True)
        stats_sb1 = sb.tile([1, 2 * H1], f32, tag="st1")
        nc.vector.tensor_copy(out=stats_sb1[:], in_=stp1[:])
        nc.sync.dma_start(out=h1_slice[NPADC : NPADC + 1, :], in_=stats_sb1[:, :H1])
        nc.sync.dma_start(out=h1_slice[NPADC + 1 : NSTR, :], in_=stats_sb1[:, H1:])
        nc.gpsimd.collective_compute(
            "AllGather", mybir.AluOpType.bypass, replica_groups=groups,
            ins=[h1_slice.opt()], outs=[h1_full.opt()],
        )

        # ---- overlapped with the AllGather: L2 z tiles + one-hots ----
        ZP2 = pp_z.tile([P, 4, KH], f32, tag="z")
        for t in range(NT):
            s0 = t * P
            zp2 = ZP2[:, t % 4, :]
            nc.tensor.matmul(
                out=zp2[:], lhsT=eaT[:, s0 : s0 + P], rhs=W1a2[:],
                start=True, stop=True,
            )
            nc.vector.tensor_scalar_max(out=z2_all[:, t, :], in0=zp2[:], scalar1=0.0)
            nc.vector.tensor_scalar(
                out=oh2_all[:, t, :], in0=iota128[:], scalar1=drel[:, t : t + 1],
                scalar2=None, op0=mybir.AluOpType.is_equal,
            )
        # ---- overlapped with the AllGather: h1^T transposes (pre-affine) ----
        h1T_raw = []
        TP = pp_u.tile([H1, 4, P], f32, tag="ut")
        for w in range(WPC):
            tp = TP[:, w % 4, :]
            nc.tensor.transpose(out=tp[:], in_=pre1[w][:], identity=ident[:])
            tr = stash.tile([H1, P], f32, tag="h1Traw")
            nc.vector.tensor_copy(out=tr[:], in_=tp[:])
            h1T_raw.append(tr)

        # ---- reduce the 8 cores' stats rows (all on partition 0) ----
        sgat = sb.tile([1, NCORES, 2 * H1], f32, tag="sgat")
        stat_rows = h1_full[:].rearrange("(c r) f -> c r f", r=NSTR)[
            :, NPADC : NPADC + 2, :
        ]
        nc.sync.dma_start(
            out=sgat[:], in_=stat_rows.rearrange("c r f -> c (r f)").unsqueeze(0)
        )
        acc = sb.tile([1, 2 * H1], f32, tag="sacc")
        nc.vector.tensor_copy(out=acc[:], in_=sgat[:, 0, :])
        for c in range(1, NCORES):
            nc.vector.tensor_add(out=acc[:], in0=acc[:], in1=sgat[:, c, :])
        # bn1 coeffs (row orientation [1, H1])
        mu = sb.tile([1, H1], f32, tag="mu")
        nc.vector.tensor_scalar_mul(out=mu[:], in0=acc[:, :H1], scalar1=1.0 / N)
        va = sb.tile([1, H1], f32, tag="va")
        nc.vector.tensor_scalar_mul(out=va[:], in0=acc[:, H1:], scalar1=1.0 / N)
        musq = sb.tile([1, H1], f32, tag="musq")
        nc.vector.tensor_mul(out=musq[:], in0=mu[:], in1=mu[:])
        nc.vector.tensor_sub(out=va[:], in0=va[:], in1=musq[:])
        nc.vector.tensor_scalar_add(out=va[:], in0=va[:], scalar1=EPS)
        sd = sb.tile([1, H1], f32, tag="sd")
        nc.scalar.activation(out=sd[:], in_=va[:], func=AF.Sqrt)
        rs = sb.tile([1, H1], f32, tag="rs")
        nc.vector.reciprocal(out=rs[:], in_=sd[:])
        sc_r = sb.tile([1, H1], f32, tag="sc_r")
        nc.vector.tensor_mul(out=sc_r[:], in0=rs[:], in1=bng1r[:])
        sh_r = sb.tile([1, H1], f32, tag="sh_r")
        nc.vector.tensor_mul(out=sh_r[:], in0=mu[:], in1=sc_r[:])
        nc.vector.tensor_sub(out=sh_r[:], in0=bnb1r[:], in1=sh_r[:])
        # broadcast [P, H1] tiles for the folded gather affine
        scp = pp_m.tile([P, H1], f32, tag="m")
        nc.tensor.matmul(out=scp[:], lhsT=onesr[:], rhs=sc_r[:], start=True, stop=True)
        scale_bc = cst.tile([P, H1], f32, tag="scale_bc")
        nc.vector.tensor_copy(out=scale_bc[:], in_=scp[:])
        shp = pp_m.tile([P, H1], f32, tag="m")
        nc.tensor.matmul(out=shp[:], lhsT=onesr[:], rhs=sh_r[:], start=True, stop=True)
        shift_bc = cst.tile([P, H1], f32, tag="shift_bc")
        nc.vector.tensor_copy(out=shift_bc[:], in_=shp[:])
        # column coeffs [H1, 1] for the transposed h1 (root2 operand)
        sccp = pp_m.tile([H1, 1], f32, tag="m")
        nc.tensor.transpose(out=sccp[:], in_=sc_r[:], identity=ident[:1, :1])
        sc_c = sb.tile([H1, 1], f32, tag="sc_c")
        nc.vector.tensor_copy(out=sc_c[:], in_=sccp[:])
        shcp = pp_m.tile([H1, 1], f32, tag="m")
        nc.tensor.transpose(out=shcp[:], in_=sh_r[:], identity=ident[:1, :1])
        sh_c = sb.tile([H1, 1], f32, tag="sh_c")
        nc.vector.tensor_copy(out=sh_c[:], in_=shcp[:])
        # normalized h1^T per window with ones row (root2+bias2 operand)
        h1T_list = []
        for w in range(WPC):
            h1Ta = stash.tile([H1 + 1, P], bf16, tag="h1Ta")
            nc.vector.tensor_scalar(
                out=h1Ta[:H1, :], in0=h1T_raw[w][:], scalar1=sc_c[:, :1],
                scalar2=sh_c[:, :1], op0=AL.mult, op1=AL.add,
            )
            nc.gpsimd.memset(h1Ta[H1 : H1 + 1, :], 1.0)
            h1T_list.append(h1Ta)

        # ======================= layer 2 =======================
        gTp[0] = pp_g.tile([P, NG], f32, tag="gtp", name="gtp")
        conv_layer(2, scale_bc, shift_bc, h1T_list)

        # bn2 stats as cols [H2, 2]
        stp2 = pp_m.tile([H2, 2], f32, tag="m")
        nc.tensor.matmul(out=stp2[:, 0:1], lhsT=acc2[:], rhs=onesP[:], start=True, stop=True)
        nc.tensor.matmul(out=stp2[:, 1:2], lhsT=acc2q[:], rhs=onesP[:], start=True, stop=True)

        # one final AllReduce carries pooled graph features + BN2 stats
        fin_sb = sb.tile([P, NG + 2], f32, tag="fin")
        nc.vector.tensor_copy(out=fin_sb[:, :NG], in_=gTp[0][:])
        nc.vector.tensor_copy(out=fin_sb[:, NG : NG + 2], in_=stp2[:])
        nc.sync.dma_start(out=fin_loc[:], in_=fin_sb[:])
        nc.gpsimd.collective_compute(
            "AllReduce", mybir.AluOpType.add, replica_groups=groups,
            ins=[fin_loc.opt()], outs=[fin_g.opt()],
        )
        fin = sb.tile([P, NG + 2], f32, tag="fin2")
        nc.sync.dma_start(out=fin[:], in_=fin_g[:])
        # bn2 coeffs (column orientation [H2, 1])
        mu2 = sb.tile([H2, 1], f32, tag="mu2")
        nc.vector.tensor_scalar_mul(out=mu2[:], in0=fin[:, NG : NG + 1], scalar1=1.0 / N)
        va2 = sb.tile([H2, 1], f32, tag="va2")
        nc.vector.tensor_scalar_mul(out=va2[:], in0=fin[:, NG + 1 : NG + 2], scalar1=1.0 / N)
        musq2 = sb.tile([H2, 1], f32, tag="musq2")
        nc.vector.tensor_mul(out=musq2[:], in0=mu2[:], in1=mu2[:])
        nc.vector.tensor_sub(out=va2[:], in0=va2[:], in1=musq2[:])
        nc.vector.tensor_scalar_add(out=va2[:], in0=va2[:], scalar1=EPS)
        sd2 = sb.tile([H2, 1], f32, tag="sd2")
        nc.scalar.activation(out=sd2[:], in_=va2[:], func=AF.Sqrt)
        rs2 = sb.tile([H2, 1], f32, tag="rs2")
        nc.vector.reciprocal(out=rs2[:], in_=sd2[:])
        sc2 = sb.tile([H2, 1], f32, tag="sc2")
        nc.vector.tensor_mul(out=sc2[:], in0=rs2[:], in1=bng2[:])
        sh2 = sb.tile([H2, 1], f32, tag="sh2")
        nc.vector.tensor_mul(out=sh2[:], in0=mu2[:], in1=sc2[:])
        nc.vector.tensor_sub(out=sh2[:], in0=bnb2[:], in1=sh2[:])
        # g = sc2 * g_raw + sh2 * gmask   (BN2 folded through the pool)
        gt = sb.tile([P, NG], f32, tag="gt")
        nc.vector.tensor_scalar_mul(out=gt[:], in0=fin[:, :NG], scalar1=sc2[:, :1])
        nc.vector.scalar_tensor_tensor(
            out=gt[:], in0=gmaskb[:], scalar=sh2[:, :1], in1=gt[:],
            op0=AL.mult, op1=AL.add,
        )

        # ======================= final MLP =======================
        l1p = pp_m.tile([H1, NG], f32, tag="m")
        nc.tensor.matmul(out=l1p[:], lhsT=l1W[:], rhs=gt[:], start=True, stop=True)
        hl = sb.tile([H1, NG], f32, tag="hl")
        nc.vector.tensor_scalar(
            out=hl[:], in0=l1p[:], scalar1=l1b[:, :1], scalar2=0.0,
            op0=AL.add, op1=AL.max,
        )
        l2p = pp_m.tile([1, NG], f32, tag="m")
        nc.tensor.matmul(out=l2p[:], lhsT=l2W[:], rhs=hl[:], start=True, stop=True)
        osb = sb.tile([1, NG], f32, tag="osb")
        nc.vector.tensor_scalar_add(out=osb[:], in0=l2p[:], scalar1=l2b[:, :1])
        nc.sync.dma_start(out=out_d[:], in_=osb[:])

    nc.compile()
    return nc


_CACHE = {}


def _get_program(T, ES):
    key = (T, ES)
    if key not in _CACHE:
        _CACHE[key] = build_program(T, ES)
    return _CACHE[key]


def make_in_maps(inputs):
    pp = _preprocess(
        inputs["x"], inputs["edge_index"], inputs["edge_attr"], inputs["batch"]
    )
    w = _weights(inputs)
    bf = ml_dtypes.bfloat16
    shared = dict(
        W1a1=w["W1a1"], W1a2=w["W1a2"],
        Wp1=w["Wp1"].astype(bf), Wp2=w["Wp2"].astype(bf),
        root1a=w["root1a"].astype(bf), root2a=w["root2a"].astype(bf),
        bng1r=w["bng1r"], bnb1r=w["bnb1r"], bng2=w["bng2"], bnb2=w["bnb2"],
        l1W=w["l1W"], l1b=w["l1b"], l2W=w["l2W"], l2b=w["l2b"],
        iota128=w["iota128"].astype(bf), iota256=w["iota256"].astype(bf),
        onesP=w["onesP"], onesr=w["onesr"],
        gmaskb=np.ascontiguousarray(
            np.broadcast_to(pp["gmask"], (P, NG)).astype(np.float32)
        ),
    )
    in_maps = []
    for c in range(NCORES):
        m = dict(shared)
        m["eaT"] = np.ascontiguousarray(pp["eaT"][c])
        m["srch"] = np.ascontiguousarray(pp["srch"][c])
        m["drel"] = np.ascontiguousarray(pp["drel"][c])
        m["icnt"] = np.ascontiguousarray(pp["icnt"][c])
        m["xsrc"] = np.ascontiguousarray(pp["xsrc"][c])
        m["xTa"] = np.ascontiguousarray(pp["xTa"][c].astype(bf))
        m["batchrel"] = np.ascontiguousarray(pp["batchrel"][c])
        m["igc"] = np.ascontiguousarray(pp["igc"][c])
        m["vmask"] = np.ascontiguousarray(pp["vmask"][c])
        in_maps.append(m)
    return in_maps, pp["T"], pp["ES"]


def _run(inputs, trace=False):
    in_maps, T, ES = make_in_maps(inputs)
    nc = _get_program(T, ES)
    res = run_bass_kernel_spmd(
        nc, in_maps, core_ids=list(range(NCORES)), trace=trace
    )
    out = np.asarray(res.results[0]["out"][0], dtype=np.float32)
    return out, res


def kernel(**inputs):
    return _run(inputs)[0]


# revision 13
# speedup vs baseline: 1.5516x; 1.2266x over previous
"""MPNN (2x NNConv + BN + global mean pool + MLP) on 8 Trainium2 cores.

Strategy (node-sharded message passing), v2:
  * Never materialize We=[E,in_c,out_c].  msg[e] = (z[e] (x) xs[e]) @ W2r
    where z=relu(ea@W1+b1), xs=x[src], W2r = reshape of W2.  Since the
    segment-sum over dst commutes with the (shared) @W2r, we scatter the
    per-edge outer products u[e]=(z (x) xs_scaled) into per-node U first,
    then do ONE matmul per node tile:  agg = U @ W2r.
  * Nodes are bin-packed into 128-node windows balanced by edge count;
    each core owns 10 windows.  Edges are grouped per window (padded to
    T tiles of 128).  Scatter = one-hot matmul on the PE accumulating
    U^T chunks in PSUM, pipelined per 128-wide chunk with the
    PSUM->SBUF cast (scalar engine) and the node matmul.
  * v2 layout changes vs v1: all per-edge tables are preloaded once as
    resident SBUF tiles (no per-tile DMAs); x[src] for layer 1 is
    gathered host-side (pure indexing) so layer 1 has no indirect DMAs;
    bias1/bias2 ride as an extra ones-row on the root operand; BN1 is
    folded into the layer-2 gather; BN2+pool commute so BN2 is applied
    after the final AllReduce.  Collectives use Shared outputs; the
    layer-2 edge-MLP z tiles / one-hots / h1^T transposes are computed
    during the h1 AllGather.
"""

import sys

import numpy as np

try:
    import concourse.bass as bass  # noqa
except Exception:  # pragma: no cover
    sys.path.insert(0, "/opt/trn_rl_repo")

import ml_dtypes
import concourse.bacc as bacc
import concourse.bass as bass
import concourse.mybir as mybir
import concourse.tile as tile
from concourse.bass import IndirectOffsetOnAxis
from concourse.bass_utils import run_bass_kernel_spmd
from concourse.masks import make_identity

P = 128
NCORES = 8
N = 10000
E = 30000
NG = 256
IN_C = 16
EDGE_C = 8
KH = 32  # edge-MLP hidden width
H1 = 64  # conv1 out channels
H2 = 128  # conv2 out channels
WPC = 10  # windows per core
NPADC = WPC * P  # padded nodes per core (1280)
NSTR = NPADC + 2  # h1 slice rows per core incl. 2 BN-stats rows
EPS = 1e-5
f32 = mybir.dt.float32
bf16 = mybir.dt.bfloat16
i32 = mybir.dt.int32

U1 = KH * IN_C  # 512
U1E = 640  # padded to 5 chunks of 128 (u | xs | zeros)
U2 = KH * H1  # 2048
U2E = 2176  # padded to 17 chunks of 128 (u | xs | zeros)


# --------------------------------------------------------------------------
# host-side preprocessing: index/layout work only
# --------------------------------------------------------------------------
def _preprocess(x, edge_index, edge_attr, batch):
    import heapq

    src = np.asarray(edge_index[0], dtype=np.int64)
    dst = np.asarray(edge_index[1], dtype=np.int64)
    deg = np.bincount(dst, minlength=N).astype(np.int64)

    # ---- bin-pack nodes into NCORES*WPC windows of exactly <=128 nodes,
    # balancing per-window edge counts (LPT greedy) ----
    NW = NCORES * WPC
    order = np.argsort(-deg, kind="stable")
    wsum = np.zeros(NW, dtype=np.int64)
    wcnt = np.zeros(NW, dtype=np.int64)
    win_of = np.empty(N, dtype=np.int64)
    slot_of = np.empty(N, dtype=np.int64)
    heap = [(0, w) for w in range(NW)]
    heapq.heapify(heap)
    for n in order:
        while True:
            _, w = heapq.heappop(heap)
            if wcnt[w] < P:
                break
        win_of[n] = w
        slot_of[n] = wcnt[w]
        wcnt[w] += 1
        wsum[w] += deg[n]
        if wcnt[w] < P:
            heapq.heappush(heap, (int(wsum[w]), w))

    T = max(1, int(-(-int(wsum.max()) // P)))  # tiles (of 128 edges) per window
    NT = WPC * T  # edge tiles per core
    ES = NT * P  # edge slots per core

    core_of = win_of // WPC
    lpos = (win_of % WPC) * P + slot_of

    # ---- per-edge placement ----
    ew = win_of[dst]
    eorder = np.argsort(ew, kind="stable")
    inv_cnt = 1.0 / np.maximum(deg, 1).astype(np.float32)

    ea_s = np.zeros((NCORES, ES, EDGE_C), dtype=np.float32)
    ones_s = np.zeros((NCORES, ES), dtype=np.float32)
    srcx_s = np.zeros((NCORES, ES), dtype=np.int64)
    srch_s = np.zeros((NCORES, ES), dtype=np.int32)
    dstrel_s = np.full((NCORES, ES), -1.0, dtype=np.float32)
    icnt_s = np.zeros((NCORES, ES), dtype=np.float32)

    ew_sorted = ew[eorder]
    starts = np.searchsorted(ew_sorted, np.arange(NW))
    ends = np.searchsorted(ew_sorted, np.arange(NW) + 1)
    ea_np = np.asarray(edge_attr, dtype=np.float32)
    for w in range(NW):
        es = eorder[starts[w] : ends[w]]
        c = w // WPC
        base = (w % WPC) * T * P
        k = len(es)
        assert k <= T * P
        sl = slice(base, base + k)
        ea_s[c, sl] = ea_np[es]
        ones_s[c, sl] = 1.0
        srcx_s[c, sl] = src[es]
        srch_s[c, sl] = (core_of[src[es]] * NSTR + lpos[src[es]]).astype(np.int32)
        dstrel_s[c, sl] = slot_of[dst[es]]
        icnt_s[c, sl] = inv_cnt[dst[es]]

    eaT = np.concatenate(
        [np.transpose(ea_s, (0, 2, 1)), ones_s[:, None, :]], axis=1
    ).astype(np.float32)

    # host gather of x[src] (pure indexing), tile-major [P, NT*IN_C]
    x_np = np.asarray(x, dtype=np.float32)
    xsrc = x_np[srcx_s.reshape(NCORES, NT, P)]  # [C, NT, P, IN_C]
    xsrc_s = np.ascontiguousarray(
        xsrc.transpose(0, 2, 1, 3).reshape(NCORES, P, NT * IN_C)
    )

    # per-edge tables in [P(slot-in-tile), NT] layout
    def t_major(a):
        return np.ascontiguousarray(
            a.reshape(NCORES, NT, P).transpose(0, 2, 1)
        )

    srch_t = t_major(srch_s)
    drel_t = t_major(dstrel_s)
    icnt_t = t_major(icnt_s)

    # ---- per-node per-core tables ----
    batch = np.asarray(batch, dtype=np.int64)
    gcnt = np.bincount(batch, minlength=NG).astype(np.int64)
    igc_node = (1.0 / np.maximum(gcnt, 1).astype(np.float32))[batch]
    gmask = (gcnt > 0).astype(np.float32)

    xTa_s = np.zeros((NCORES, IN_C + 1, NPADC), dtype=np.float32)
    xTa_s[:, IN_C, :] = 1.0  # ones row for fused bias
    batch_s = np.full((NCORES, P, WPC), -1.0, dtype=np.float32)
    igc_s = np.zeros((NCORES, P, WPC), dtype=np.float32)
    vmask_s = np.zeros((NCORES, P, WPC), dtype=np.float32)
    for c in range(NCORES):
        m = core_of == c
        lp = lpos[m]
        xTa_s[c][:IN_C, lp] = x_np[m].T
        batch_s[c][lp % P, lp // P] = batch[m].astype(np.float32)
        igc_s[c][lp % P, lp // P] = igc_node[m]
        vmask_s[c][lp % P, lp // P] = 1.0

    return dict(
        T=T, ES=ES, NT=NT, eaT=eaT, srch=srch_t, drel=drel_t, icnt=icnt_t,
        xsrc=xsrc_s, xTa=xTa_s, batchrel=batch_s, igc=igc_s, vmask=vmask_s,
        gmask=gmask,
    )


def _weights(p):
    w = {}
    w["W1a1"] = np.concatenate([p["nn1_W1"], p["nn1_b1"][None, :]], 0).astype(np.float32)
    w["W1a2"] = np.concatenate([p["nn2_W1"], p["nn2_b1"][None, :]], 0).astype(np.float32)
    wp1 = np.zeros((U1E, H1), dtype=np.float32)
    wp1[:U1] = p["nn1_W2"].reshape(KH, IN_C, H1).reshape(U1, H1)
    wp1[U1 : U1 + IN_C] = p["nn1_b2"].reshape(IN_C, H1)
    w["Wp1"] = wp1
    wp2 = np.zeros((U2E, H2), dtype=np.float32)
    wp2[:U2] = p["nn2_W2"].reshape(KH, H1, H2).reshape(U2, H2)
    wp2[U2 : U2 + H1] = p["nn2_b2"].reshape(H1, H2)
    w["Wp2"] = wp2
    w["root1a"] = np.concatenate(
        [np.asarray(p["root1"], np.float32), np.asarray(p["bias1"], np.float32)[None, :]], 0
    )
    w["root2a"] = np.concatenate(
        [np.asarray(p["root2"], np.float32), np.asarray(p["bias2"], np.float32)[None, :]], 0
    )
    w["bng1r"] = np.asarray(p["bn1_g"], np.float32)[None, :]
    w["bnb1r"] = np.asarray(p["bn1_b"], np.float32)[None, :]
    w["bng2"] = np.asarray(p["bn2_g"], np.float32)[:, None]
    w["bnb2"] = np.asarray(p["bn2_b"], np.float32)[:, None]
    w["l1W"] = np.asarray(p["lin1_W"], np.float32)
    w["l1b"] = np.asarray(p["lin1_b"], np.float32)[:, None]
    w["l2W"] = np.asarray(p["lin2_W"], np.float32)
    w["l2b"] = np.asarray(p["lin2_b"], np.float32)[None, :]
    w["iota128"] = np.broadcast_to(np.arange(P, dtype=np.float32), (P, P)).copy()
    w["iota256"] = np.broadcast_to(np.arange(NG, dtype=np.float32), (P, NG)).copy()
    w["onesP"] = np.ones((P, 1), dtype=np.float32)
    w["onesr"] = np.ones((1, P), dtype=np.float32)
    return w


# --------------------------------------------------------------------------
# device program (identical for all cores; per-core data comes via inputs)
# --------------------------------------------------------------------------
def build_program(T, ES):
    NT = WPC * T
    AL = mybir.AluOpType
    AF = mybir.ActivationFunctionType
    nc = bacc.Bacc("TRN2", target_bir_lowering=False, debug=False, num_devices=NCORES)

    def din(name, shape, dtype=f32):
        return nc.dram_tensor(name, shape, dtype, kind="ExternalInput").ap()

    eaT_d = din("eaT", [EDGE_C + 1, ES])
    srch_d = din("srch", [P, NT], i32)
    drel_d = din("drel", [P, NT])
    icnt_d = din("icnt", [P, NT])
    xsrc_d = din("xsrc", [P, NT * IN_C])
    xTa_d = din("xTa", [IN_C + 1, NPADC], bf16)
    batch_d = din("batchrel", [P, WPC])
    igc_d = din("igc", [P, WPC])
    vmask_d = din("vmask", [P, WPC])
    W1a1_d = din("W1a1", [EDGE_C + 1, KH])
    W1a2_d = din("W1a2", [EDGE_C + 1, KH])
    Wp1_d = din("Wp1", [U1E, H1], bf16)
    Wp2_d = din("Wp2", [U2E, H2], bf16)
    root1a_d = din("root1a", [IN_C + 1, H1], bf16)
    root2a_d = din("root2a", [H1 + 1, H2], bf16)
    bng1r_d = din("bng1r", [1, H1])
    bnb1r_d = din("bnb1r", [1, H1])
    bng2_d = din("bng2", [H2, 1])
    bnb2_d = din("bnb2", [H2, 1])
    l1W_d = din("l1W", [H2, H1])
    l1b_d = din("l1b", [H1, 1])
    l2W_d = din("l2W", [H1, 1])
    l2b_d = din("l2b", [1, 1])
    iota128_d = din("iota128", [P, P], bf16)
    iota256_d = din("iota256", [P, NG], bf16)
    onesP_d = din("onesP", [P, 1])
    onesr_d = din("onesr", [1, P])
    gmaskb_d = din("gmaskb", [P, NG])
    out_d = nc.dram_tensor("out", [1, NG], f32, kind="ExternalOutput").ap()

    NC1 = U1E // P  # 5 chunks
    NC2 = U2E // P  # 17 chunks

    from contextlib import ExitStack

    with tile.TileContext(nc) as tc, ExitStack() as pools:
        cst = pools.enter_context(tc.tile_pool(name="cst", bufs=1))
        sb = pools.enter_context(tc.tile_pool(name="sb", bufs=3))
        stash = pools.enter_context(tc.tile_pool(name="stash", bufs=WPC))
        pp_u = pools.enter_context(tc.tile_pool(name="pp_u", bufs=1, space="PSUM"))
        pp_pre = pools.enter_context(tc.tile_pool(name="pp_pre", bufs=1, space="PSUM"))
        pp_z = pools.enter_context(tc.tile_pool(name="pp_z", bufs=1, space="PSUM"))
        pp_m = pools.enter_context(tc.tile_pool(name="pp_m", bufs=2, space="PSUM"))
        pp_g = pools.enter_context(tc.tile_pool(name="pp_g", bufs=1, space="PSUM"))
        dram = pools.enter_context(tc.tile_pool(name="dram", bufs=1, space="DRAM"))

        # ---- resident constants (spread initial DMAs across engine queues) ----
        _eng = [nc.sync, nc.scalar, nc.gpsimd]
        _ei = [0]

        def load(shape, ap, name, dt=f32):
            t = cst.tile(shape, dt, tag=name, name=name)
            _eng[_ei[0] % len(_eng)].dma_start(out=t[:], in_=ap)
            _ei[0] += 1
            return t

        ident = cst.tile([P, P], f32, tag="ident")
        make_identity(nc, ident[:])
        W1a1 = load([EDGE_C + 1, KH], W1a1_d[:], "W1a1")
        W1a2 = load([EDGE_C + 1, KH], W1a2_d[:], "W1a2")
        Wp1 = cst.tile([P, NC1, H1], bf16, tag="Wp1")
        nc.sync.dma_start(out=Wp1[:], in_=Wp1_d.rearrange("(c p) o -> p c o", p=P))
        Wp2 = cst.tile([P, NC2, H2], bf16, tag="Wp2")
        nc.scalar.dma_start(out=Wp2[:], in_=Wp2_d.rearrange("(c p) o -> p c o", p=P))
        eaT = load([EDGE_C + 1, ES], eaT_d[:], "eaT")
        srch = load([P, NT], srch_d[:], "srch", i32)
        drel = load([P, NT], drel_d[:], "drel")
        icnt = load([P, NT], icnt_d[:], "icnt")
        xsrc = load([P, NT * IN_C], xsrc_d[:], "xsrc")
        xTa = load([IN_C + 1, NPADC], xTa_d[:], "xTa", bf16)
        root1a = load([IN_C + 1, H1], root1a_d[:], "root1a", bf16)
        root2a = load([H1 + 1, H2], root2a_d[:], "root2a", bf16)
        bng1r = load([1, H1], bng1r_d[:], "bng1r")
        bnb1r = load([1, H1], bnb1r_d[:], "bnb1r")
        bng2 = load([H2, 1], bng2_d[:], "bng2")
        bnb2 = load([H2, 1], bnb2_d[:], "bnb2")
        l1W = load([H2, H1], l1W_d[:], "l1W")
        l1b = load([H1, 1], l1b_d[:], "l1b")
        l2W = load([H1, 1], l2W_d[:], "l2W")
        l2b = load([1, 1], l2b_d[:], "l2b")
        iota128 = load([P, P], iota128_d[:], "iota128", bf16)
        iota256 = load([P, NG], iota256_d[:], "iota256", bf16)
        onesP = load([P, 1], onesP_d[:], "onesP")
        onesr = load([1, P], onesr_d[:], "onesr")
        gmaskb = load([P, NG], gmaskb_d[:], "gmaskb")
        vmask = load([P, WPC], vmask_d[:], "vmask")
        igc = load([P, WPC], igc_d[:], "igc")
        batchrel = load([P, WPC], batch_d[:], "batchrel")

        # stats accumulators (SBUF, f32)
        acc1 = cst.tile([P, H1], f32, tag="acc1")
        acc1q = cst.tile([P, H1], f32, tag="acc1q")
        acc2 = cst.tile([P, H2], f32, tag="acc2")
        acc2q = cst.tile([P, H2], f32, tag="acc2q")
        for a in (acc1, acc1q, acc2, acc2q):
            nc.vector.memset(a[:], 0.0)

        h1_slice = dram.tile([NSTR, H1], f32, tag="h1s")
        h1_full = dram.tile([NCORES * NSTR, H1], f32, tag="h1f", addr_space="Shared")
        fin_loc = dram.tile([P, NG + 2], bf16, tag="finl")
        fin_g = dram.tile([P, NG + 2], bf16, tag="fing", addr_space="Shared")

        groups = [list(range(NCORES))]
        gTp = [None]

        # stash for L2 precomputed per-tile z (bf16) and one-hots (bf16)
        z2_all = cst.tile([P, NT, KH], bf16, tag="z2all")
        oh2_all = cst.tile([P, NT, P], bf16, tag="oh2all")

        # rotating u slabs with the zero tails initialized ONCE
        u1_slab = cst.tile([P, 6, U1E], bf16, tag="u1slab")
        u2_slab = cst.tile([P, 6, U2E], bf16, tag="u2slab")
        for si in range(6):
            nc.gpsimd.memset(u1_slab[:, si, U1 + IN_C :], 0.0)
            nc.gpsimd.memset(u2_slab[:, si, U2 + H1 :], 0.0)

        # ================= generic conv layer =================
        def conv_layer(layer, scale_bc=None, shift_bc=None, h1T_list=None):
            if layer == 1:
                inc, outc, UE, NCH, W1a, Wp = IN_C, H1, U1E, NC1, W1a1, Wp1
                uw, acc, accq = U1, acc1, acc1q
            else:
                inc, outc, UE, NCH, W1a, Wp = H1, H2, U2E, NC2, W1a2, Wp2
                uw, acc, accq = U2, acc2, acc2q
            pre_list = []
            # slot-rotated PSUM tiles (PSUM slots are bank-granular)
            UT = pp_u.tile([P, 4, P], f32, tag="ut")
            PRE = pp_pre.tile([P, 4, P], f32, tag="pre")
            ZP = pp_z.tile([P, 4, KH], f32, tag="z")
            for w in range(WPC):
                u_tiles = []
                oh_tiles = []
                for t3 in range(T):
                    t = w * T + t3
                    s0 = t * P
                    if layer == 1:
                        # z = relu(ea @ W1 + b1)  (bf16 out)
                        zp = ZP[:, t % 4, :]
                        nc.tensor.matmul(
                            out=zp[:], lhsT=eaT[:, s0 : s0 + P], rhs=W1a[:],
                            start=True, stop=True,
                        )
                        z = sb.tile([P, KH], bf16, tag="z_sb", bufs=4)
                        nc.vector.tensor_scalar_max(out=z[:], in0=zp[:], scalar1=0.0)
                        oh = sb.tile([P, P], bf16, tag="oh", bufs=4)
                        nc.vector.tensor_scalar(
                            out=oh[:], in0=iota128[:], scalar1=drel[:, t : t + 1],
                            scalar2=None, op0=AL.is_equal,
                        )
                        xss = sb.tile([P, inc], bf16, tag="xss1", bufs=4)
                        nc.vector.tensor_scalar_mul(
                            out=xss[:], in0=xsrc[:, t * inc : (t + 1) * inc],
                            scalar1=icnt[:, t : t + 1],
                        )
                    else:
                        z = z2_all[:, t, :]
                        oh = oh2_all[:, t, :]
                        # gather xs rows from the AllGathered h1
                        xs = sb.tile([P, inc], f32, tag="xs", bufs=4)
                        nc.gpsimd.indirect_dma_start(
                            out=xs[:],
                            out_offset=None,
                            in_=h1_full[:],
                            in_offset=IndirectOffsetOnAxis(ap=srch[:, t : t + 1], axis=0),
                        )
                        # xss = (xs*icnt)*bn_scale + bn_shift*icnt  (bf16 out)
                        xsi = sb.tile([P, inc], f32, tag="xsi", bufs=3)
                        nc.vector.scalar_tensor_tensor(
                            out=xsi[:], in0=xs[:], scalar=icnt[:, t : t + 1],
                            in1=scale_bc[:], op0=AL.mult, op1=AL.mult,
                        )
                        xss = sb.tile([P, inc], bf16, tag="xss2", bufs=4)
                        nc.vector.scalar_tensor_tensor(
                            out=xss[:], in0=shift_bc[:], scalar=icnt[:, t : t + 1],
                            in1=xsi[:], op0=AL.mult, op1=AL.add,
                        )

                    # u = [z (x) xss | xss | zero-tail (slab preset)]
                    u = (u1_slab if layer == 1 else u2_slab)[:, t % 6, :]
                    nc.vector.tensor_tensor(
                        out=u[:, :uw].rearrange("p (k i) -> p k i", k=KH),
                        in0=z.unsqueeze(2).to_broadcast([P, KH, inc]),
                        in1=xss.unsqueeze(1).to_broadcast([P, KH, inc]),
                        op=AL.mult,
                    )
                    nc.scalar.copy(out=u[:, uw : uw + inc], in_=xss[:])
                    u_tiles.append(u)
                    oh_tiles.append(oh)

                # scatter / pair-cast / node matmul pipelined, node lags by 2
                pre = PRE[:, w % 4, :outc]
                uts_pairs = []  # (UTs_tile, c0, n_in_pair)
                nodes_done = [0]

                def emit_node(cn):
                    nc.tensor.matmul(
                        out=pre[:], lhsT=uts_pairs[cn // 2][0][:, cn % 2, :],
                        rhs=Wp[:, cn, :outc],
                        start=(cn == 0), stop=False,
                    )

                for c in range(NCH):
                    UTc = UT[:, c % 4, :]
                    for t3 in range(T):
                        nc.tensor.matmul(
                            out=UTc[:],
                            lhsT=u_tiles[t3][:, c * P : (c + 1) * P],
                            rhs=oh_tiles[t3][:],
                            start=(t3 == 0),
                            stop=(t3 == T - 1),
                        )
                    if c % 2 == 1:
                        # cast the finished pair of chunks in one op
                        UTs = sb.tile([P, 2, P], bf16, tag="uts", bufs=4)
                        nc.scalar.copy(out=UTs[:], in_=UT[:, (c - 1) % 4 : (c - 1) % 4 + 2, :])
                        uts_pairs.append((UTs, c - 1, 2))
                    elif c == NCH - 1:
                        UTs = sb.tile([P, 2, P], bf16, tag="uts", bufs=4)
                        nc.scalar.copy(out=UTs[:, 0, :], in_=UTc[:])
                        uts_pairs.append((UTs, c, 1))
                    # node matmuls lag two chunks behind the scatter
                    while nodes_done[0] <= c - 2:
                        emit_node(nodes_done[0])
                        nodes_done[0] += 1
                while nodes_done[0] < NCH:
                    emit_node(nodes_done[0])
                    nodes_done[0] += 1
                # root + bias (ones row) closes the accumulation
                if layer == 1:
                    nc.tensor.matmul(
                        out=pre[:], lhsT=xTa[:, w * P : (w + 1) * P], rhs=root1a[:],
                        start=False, stop=True,
                    )
                else:
                    nc.tensor.matmul(
                        out=pre[:], lhsT=h1T_list[w][:], rhs=root2a[:],
                        start=False, stop=True,
                    )
                # relu then mask out pad nodes
                if layer == 1:
                    pre_sb = stash.tile([P, outc], f32, tag="pre1")
                else:
                    pre_sb = sb.tile([P, outc], f32, tag="pre2")
                nc.vector.tensor_scalar(
                    out=pre_sb[:], in0=pre[:], scalar1=0.0,
                    scalar2=vmask[:, w : w + 1], op0=AL.max, op1=AL.mult,
                )
                sq = sb.tile([P, outc], f32, tag="sq")
                nc.scalar.activation(out=sq[:], in_=pre_sb[:], func=AF.Square)
                nc.vector.tensor_add(out=acc[:, :outc], in0=acc[:, :outc], in1=pre_sb[:])
                nc.vector.tensor_add(out=accq[:, :outc], in0=accq[:, :outc], in1=sq[:])
                if layer == 1:
                    pre_list.append(pre_sb)
                    # pre-BN h1 rows go straight out for the AllGather
                    nc.sync.dma_start(
                        out=h1_slice[w * P : (w + 1) * P, :], in_=pre_sb[:]
                    )
                else:
                    # pool the un-normalized output (BN2 applied post-AR)
                    h2p = sb.tile([P, H2], bf16, tag="h2p")
                    nc.vector.tensor_scalar_mul(out=h2p[:], in0=pre_sb[:], scalar1=igc[:, w : w + 1])
                    ohg = sb.tile([P, NG], bf16, tag="ohg")
                    nc.vector.tensor_scalar(
                        out=ohg[:], in0=iota256[:], scalar1=batchrel[:, w : w + 1],
                        scalar2=None, op0=AL.is_equal,
                    )
                    nc.tensor.matmul(
                        out=gTp[0][:], lhsT=h2p[:], rhs=ohg[:],
                        start=(w == 0), stop=(w == WPC - 1),
                    )
            return pre_list

        # ======================= layer 1 =======================
        pre1 = conv_layer(1)
        # bn1 stats: [1, 2*H1] = [sum | sumsq] reduced over partitions
        stp1 = pp_m.tile([1, 2 * H1], f32, tag="m")
        nc.tensor.matmul(out=stp1[0:1, :H1], lhsT=onesP[:], rhs=acc1[:], start=True, stop=True)
        nc.tensor.matmul(out=stp1[0:1, H1:], lhsT=onesP[:], rhs=acc1q[:], start=True, stop=True)
        stats_sb1 = sb.tile([1, 2 * H1], f32, tag="st1")
        nc.vector.tensor_copy(out=stats_sb1[:], in_=stp1[:])
        nc.sync.dma_start(out=h1_slice[NPADC : NPADC + 1, :], in_=stats_sb1[:, :H1])
        nc.sync.dma_start(out=h1_slice[NPADC + 1 : NSTR, :], in_=stats_sb1[:, H1:])
        nc.gpsimd.collective_compute(
            "AllGather", mybir.AluOpType.bypass, replica_groups=groups,
            ins=[h1_slice.opt()], outs=[h1_full.opt()],
        )

        # ---- overlapped with the AllGather: L2 z tiles + one-hots ----
        ZP2 = pp_z.tile([P, 4, KH], f32, tag="z")
        for t in range(NT):
            s0 = t * P
            zp2 = ZP2[:, t % 4, :]
            nc.tensor.matmul(
                out=zp2[:], lhsT=eaT[:, s0 : s0 + P], rhs=W1a2[:],
                start=True, stop=True,
            )
            nc.vector.tensor_scalar_max(out=z2_all[:, t, :], in0=zp2[:], scalar1=0.0)
            nc.vector.tensor_scalar(
                out=oh2_all[:, t, :], in0=iota128[:], scalar1=drel[:, t : t + 1],
                scalar2=None, op0=mybir.AluOpType.is_equal,
            )
        # ---- overlapped with the AllGather: h1^T transposes (pre-affine) ----
        h1T_raw = []
        TP = pp_u.tile([H1, 4, P], f32, tag="ut")
        for w in range(WPC):
            tp = TP[:, w % 4, :]
            nc.tensor.transpose(out=tp[:], in_=pre1[w][:], identity=ident[:])
            tr = stash.tile([H1, P], f32, tag="h1Traw")
            nc.vector.tensor_copy(out=tr[:], in_=tp[:])
            h1T_raw.append(tr)

        # ---- reduce the 8 cores' stats rows (all on partition 0) ----
        sgat = sb.tile([1, NCORES, 2 * H1], f32, tag="sgat")
        stat_rows = h1_full[:].rearrange("(c r) f -> c r f", r=NSTR)[
            :, NPADC : NPADC + 2, :
        ]
        nc.sync.dma_start(
            out=sgat[:], in_=stat_rows.rearrange("c r f -> c (r f)").unsqueeze(0)
        )
        acc = sb.tile([1, 2 * H1], f32, tag="sacc")
        nc.vector.tensor_copy(out=acc[:], in_=sgat[:, 0, :])
        for c in range(1, NCORES):
            nc.vector.tensor_add(out=acc[:], in0=acc[:], in1=sgat[:, c, :])
        # bn1 coeffs (row orientation [1, H1])
        mu = sb.tile([1, H1], f32, tag="mu")
        nc.vector.tensor_scalar_mul(out=mu[:], in0=acc[:, :H1], scalar1=1.0 / N)
        va = sb.tile([1, H1], f32, tag="va")
        nc.vector.tensor_scalar_mul(out=va[:], in0=acc[:, H1:], scalar1=1.0 / N)
        musq = sb.tile([1, H1], f32, tag="musq")
        nc.vector.tensor_mul(out=musq[:], in0=mu[:], in1=mu[:])
        nc.vector.tensor_sub(out=va[:], in0=va[:], in1=musq[:])
        nc.vector.tensor_scalar_add(out=va[:], in0=va[:], scalar1=EPS)
        sd = sb.tile([1, H1], f32, tag="sd")
        nc.scalar.activation(out=sd[:], in_=va[:], func=AF.Sqrt)
        rs = sb.tile([1, H1], f32, tag="rs")
        nc.vector.reciprocal(out=rs[:], in_=sd[:])
        sc_r = sb.tile([1, H1], f32, tag="sc_r")
        nc.vector.tensor_mul(out=sc_r[:], in0=rs[:], in1=bng1r[:])
        sh_r = sb.tile([1, H1], f32, tag="sh_r")
        nc.vector.tensor_mul(out=sh_r[:], in0=mu[:], in1=sc_r[:])
        nc.vector.tensor_sub(out=sh_r[:], in0=bnb1r[:], in1=sh_r[:])
        # broadcast [P, H1] tiles for the folded gather affine
        scp = pp_m.tile([P, H1], f32, tag="m")
        nc.tensor.matmul(out=scp[:], lhsT=onesr[:], rhs=sc_r[:], start=True, stop=True)
        scale_bc = cst.tile([P, H1], f32, tag="scale_bc")
        nc.vector.tensor_copy(out=scale_bc[:], in_=scp[:])
        shp = pp_m.tile([P, H1], f32, tag="m")
        nc.tensor.matmul(out=shp[:], lhsT=onesr[:], rhs=sh_r[:], start=True, stop=True)
        shift_bc = cst.tile([P, H1], f32, tag="shift_bc")
        nc.vector.tensor_copy(out=shift_bc[:], in_=shp[:])
        # column coeffs [H1, 1] for the transposed h1 (root2 operand)
        sccp = pp_m.tile([H1, 1], f32, tag="m")
        nc.tensor.transpose(out=sccp[:], in_=sc_r[:], identity=ident[:1, :1])
        sc_c = sb.tile([H1, 1], f32, tag="sc_c")
        nc.vector.tensor_copy(out=sc_c[:], in_=sccp[:])
        shcp = pp_m.tile([H1, 1], f32, tag="m")
        nc.tensor.transpose(out=shcp[:], in_=sh_r[:], identity=ident[:1, :1])
        sh_c = sb.tile([H1, 1], f32, tag="sh_c")
        nc.vector.tensor_copy(out=sh_c[:], in_=shcp[:])
        # normalized h1^T per window with ones row (root2+bias2 operand)
        h1T_list = []
        for w in range(WPC):
            h1Ta = stash.tile([H1 + 1, P], bf16, tag="h1Ta")
            nc.vector.tensor_scalar(
                out=h1Ta[:H1, :], in0=h1T_raw[w][:], scalar1=sc_c[:, :1],
                scalar2=sh_c[:, :1], op0=AL.mult, op1=AL.add,
            )
            nc.gpsimd.memset(h1Ta[H1 : H1 + 1, :], 1.0)
            h1T_list.append(h1Ta)

        # ======================= layer 2 =======================
        gTp[0] = pp_g.tile([P, NG], f32, tag="gtp", name="gtp")
        conv_layer(2, scale_bc, shift_bc, h1T_list)

        # bn2 stats as cols [H2, 2]
        stp2 = pp_m.tile([H2, 2], f32, tag="m")
        nc.tensor.matmul(out=stp2[:, 0:1], lhsT=acc2[:], rhs=onesP[:], start=True, stop=True)
        nc.tensor.matmul(out=stp2[:, 1:2], lhsT=acc2q[:], rhs=onesP[:], start=True, stop=True)

        # one final AllReduce carries pooled graph features + BN2 stats
        fin_sb = sb.tile([P, NG + 2], bf16, tag="fin")
        nc.vector.tensor_copy(out=fin_sb[:, :NG], in_=gTp[0][:])
        nc.vector.tensor_copy(out=fin_sb[:, NG : NG + 2], in_=stp2[:])
        nc.sync.dma_start(out=fin_loc[:], in_=fin_sb[:])
        nc.gpsimd.collective_compute(
            "AllReduce", mybir.AluOpType.add, replica_groups=groups,
            ins=[fin_loc.opt()], outs=[fin_g.opt()],
        )
        fin = sb.tile([P, NG + 2], bf16, tag="fin2")
        nc.sync.dma_start(out=fin[:], in_=fin_g[:])
        # bn2 coeffs (column orientation [H2, 1])
        mu2 = sb.tile([H2, 1], f32, tag="mu2")
        nc.vector.tensor_scalar_mul(out=mu2[:], in0=fin[:, NG : NG + 1], scalar1=1.0 / N)
        va2 = sb.tile([H2, 1], f32, tag="va2")
        nc.vector.tensor_scalar_mul(out=va2[:], in0=fin[:, NG + 1 : NG + 2], scalar1=1.0 / N)
        musq2 = sb.tile([H2, 1], f32, tag="musq2")
        nc.vector.tensor_mul(out=musq2[:], in0=mu2[:], in1=mu2[:])
        nc.vector.tensor_sub(out=va2[:], in0=va2[:], in1=musq2[:])
        nc.vector.tensor_scalar_add(out=va2[:], in0=va2[:], scalar1=EPS)
        sd2 = sb.tile([H2, 1], f32, tag="sd2")
        nc.scalar.activation(out=sd2[:], in_=va2[:], func=AF.Sqrt)
        rs2 = sb.tile([H2, 1], f32, tag="rs2")
        nc.vector.reciprocal(out=rs2[:], in_=sd2[:])
        sc2 = sb.tile([H2, 1], f32, tag="sc2")
        nc.vector.tensor_mul(out=sc2[:], in0=rs2[:], in1=bng2[:])
        sh2 = sb.tile([H2, 1], f32, tag="sh2")
        nc.vector.tensor_mul(out=sh2[:], in0=mu2[:], in1=sc2[:])
        nc.vector.tensor_sub(out=sh2[:], in0=bnb2[:], in1=sh2[:])
        # g = sc2 * g_raw + sh2 * gmask   (BN2 folded through the pool)
        gt = sb.tile([P, NG], f32, tag="gt")
        nc.vector.tensor_scalar_mul(out=gt[:], in0=fin[:, :NG], scalar1=sc2[:, :1])
        nc.vector.scalar_tensor_tensor(
            out=gt[:], in0=gmaskb[:], scalar=sh2[:, :1], in1=gt[:],
            op0=AL.mult, op1=AL.add,
        )

        # ======================= final MLP =======================
        l1p = pp_m.tile([H1, NG], f32, tag="m")
        nc.tensor.matmul(out=l1p[:], lhsT=l1W[:], rhs=gt[:], start=True, stop=True)
        hl = sb.tile([H1, NG], f32, tag="hl")
        nc.vector.tensor_scalar(
            out=hl[:], in0=l1p[:], scalar1=l1b[:, :1], scalar2=0.0,
            op0=AL.add, op1=AL.max,
        )
        l2p = pp_m.tile([1, NG], f32, tag="m")
        nc.tensor.matmul(out=l2p[:], lhsT=l2W[:], rhs=hl[:], start=True, stop=True)
        osb = sb.tile([1, NG], f32, tag="osb")
        nc.vector.tensor_scalar_add(out=osb[:], in0=l2p[:], scalar1=l2b[:, :1])
        nc.sync.dma_start(out=out_d[:], in_=osb[:])

    nc.compile()
    return nc


_CACHE = {}


def _get_program(T, ES):
    key = (T, ES)
    if key not in _CACHE:
        _CACHE[key] = build_program(T, ES)
    return _CACHE[key]


def make_in_maps(inputs):
    pp = _preprocess(
        inputs["x"], inputs["edge_index"], inputs["edge_attr"], inputs["batch"]
    )
    w = _weights(inputs)
    bf = ml_dtypes.bfloat16
    shared = dict(
        W1a1=w["W1a1"], W1a2=w["W1a2"],
        Wp1=w["Wp1"].astype(bf), Wp2=w["Wp2"].astype(bf),
        root1a=w["root1a"].astype(bf), root2a=w["root2a"].astype(bf),
        bng1r=w["bng1r"], bnb1r=w["bnb1r"], bng2=w["bng2"], bnb2=w["bnb2"],
        l1W=w["l1W"], l1b=w["l1b"], l2W=w["l2W"], l2b=w["l2b"],
        iota128=w["iota128"].astype(bf), iota256=w["iota256"].astype(bf),
        onesP=w["onesP"], onesr=w["onesr"],
        gmaskb=np.ascontiguousarray(
            np.broadcast_to(pp["gmask"], (P, NG)).astype(np.float32)
        ),
    )
    in_maps = []
    for c in range(NCORES):
        m = dict(shared)
        m["eaT"] = np.ascontiguousarray(pp["eaT"][c])
        m["srch"] = np.ascontiguousarray(pp["srch"][c])
        m["drel"] = np.ascontiguousarray(pp["drel"][c])
        m["icnt"] = np.ascontiguousarray(pp["icnt"][c])
        m["xsrc"] = np.ascontiguousarray(pp["xsrc"][c])
        m["xTa"] = np.ascontiguousarray(pp["xTa"][c].astype(bf))
        m["batchrel"] = np.ascontiguousarray(pp["batchrel"][c])
        m["igc"] = np.ascontiguousarray(pp["igc"][c])
        m["vmask"] = np.ascontiguousarray(pp["vmask"][c])
        in_maps.append(m)
    return in_maps, pp["T"], pp["ES"]


def _run(inputs, trace=False):
    in_maps, T, ES = make_in_maps(inputs)
    nc = _get_program(T, ES)
    res = run_bass_kernel_spmd(
        nc, in_maps, core_ids=list(range(NCORES)), trace=trace
    )
    out = np.asarray(res.results[0]["out"][0], dtype=np.float32)
    return out, res


def kernel(**inputs):
    return _run(inputs)[0]
